# revision 1
# baseline (speedup 1.0000x reference)
"""Trainium2 Bass kernel for nn_ComposerModule (dense_transformer), v2.

Data-parallel over batch: 32 batch items -> 8 NeuronCores, 4 per core.

Key algebraic restructure vs v1: the per-layer v-projection is folded away.
  op_out[b,o,h] = sum_s v[b,s,h] * relw[b,s,o] * ops[o,s]
               = (w @ x) @ Wv^T + (sum_s w) bv      with w[o,s]=relw[s,o]*ops[o,s]
so the dominant [S,H]x[H,H] matmul per (layer,batch) becomes [O,S]x[S,H] +
[O,H]x[H,H] (~20x fewer FLOPs).  Both softmaxes share one exp(logits):
logits absmax ~2 so max-subtraction is skipped; relw row-normalizes exp(lg)
over s (DVE scalar_tensor_tensor fused with the operators product and the
row-sum for the bias term), opw column-normalizes over o (GPSIMD
partition_all_reduce + DVE divide).

The residual stream x is kept bf16 in BOTH orientations:
  xt[b]: [128(h), 8, S]  master, updated in-place by the residual add
  xn[b]: [128(s), 4, H]  regenerated each layer from xt via HWDGE xbar
                         DMA-transposes (8 per batch, verified mapping)
Embedding: non-transposed SWDGE row gather lands directly in xn layout;
pe added on DVE; xt0 via the inverse DMA-transposes.  oq/oqk/c and bout are
folded on the host (weight-only preprocessing).  oo is batched over the 4
batch items (M=64) and un-batched with identity-slice "select" matmuls
(engines cannot move data across partitions; matmul bases must be 0/32/64).

The layer body is emitted STAGE-major (all batches per stage): the
per-engine instruction order is fixed at schedule time, so batch-major
emission head-of-line-blocks each engine queue on the previous batch's
serial softmax chain.
"""
import math

import numpy as np
import ml_dtypes

B, S, H, O, V, OUT, L = 32, 512, 1024, 16, 32000, 1000, 4
NCORES = 8
BPC = B // NCORES
BF16 = ml_dtypes.bfloat16

_cache = {}


def _sinusoidal_pos_emb(seq_len, dim):
    pos = np.arange(seq_len)[:, None].astype(np.float32)
    div = np.exp(np.arange(0, dim, 2).astype(np.float32) * (-math.log(10000.0) / dim))
    pe = np.zeros((seq_len, dim), dtype=np.float32)
    pe[:, 0::2] = np.sin(pos * div)
    pe[:, 1::2] = np.cos(pos * div)
    return pe


def _build_program():
    import concourse.bacc as bacc
    import concourse.bass as bass
    import concourse.tile as tile
    from concourse import bass_isa
    from concourse import mybir

    dt = mybir.dt
    f32, bf16, i16 = dt.float32, dt.bfloat16, dt.int16
    PSUM = bass.MemorySpace.PSUM
    Alu = mybir.AluOpType
    Act = mybir.ActivationFunctionType

    nc = bacc.Bacc("TRN2", target_bir_lowering=False, debug=False, num_devices=NCORES)

    emb_d = nc.declare_dram_parameter("emb", [V, H], bf16, isOutput=False)
    tok_d = nc.declare_dram_parameter("tok", [BPC, 128, S // 16], i16, isOutput=False)
    pen_d = nc.declare_dram_parameter("pen", [128, 4, H], bf16, isOutput=False)
    oqkt_d = nc.declare_dram_parameter("oqkt", [128, 8, O], bf16, isOutput=False)
    c_d = nc.declare_dram_parameter("c", [O, 1], f32, isOutput=False)
    wvt_d = nc.declare_dram_parameter("wvt", [128, 8, H], bf16, isOutput=False)
    wot_d = nc.declare_dram_parameter("wot", [128, 8, OUT], bf16, isOutput=False)
    ops_d = nc.declare_dram_parameter("ops", [O, S], bf16, isOutput=False)
    i64b_d = nc.declare_dram_parameter("idn64", [64, 64], bf16, isOutput=False)
    i128b_d = nc.declare_dram_parameter("idn128", [128, 128], bf16, isOutput=False)
    bvb_d = nc.declare_dram_parameter("bvb", [16, H], bf16, isOutput=False)
    out_d = nc.declare_dram_parameter("out", [BPC, 4, 128, OUT], bf16, isOutput=True)

    with tile.TileContext(nc) as tc:
        with (
            tc.tile_pool(name="wts", bufs=1) as wp,
            tc.tile_pool(name="xres", bufs=1) as xp,
            tc.tile_pool(name="work", bufs=2) as wk,
            tc.tile_pool(name="sm", bufs=3) as sm,
            tc.tile_pool(name="psbig", bufs=3, space=PSUM) as psA,   # [128,512]
            tc.tile_pool(name="pssm", bufs=2, space=PSUM) as psS,    # [16,512]
            tc.tile_pool(name="psoo", bufs=1, space=PSUM) as psO,    # [64,512]
        ):
            # ---- persistent weights
            oqkt = wp.tile([128, 8, O], bf16)
            c_sb = wp.tile([O, 1], f32)
            wvt = wp.tile([128, 8, H], bf16)
            wot = wp.tile([128, 8, OUT], bf16)
            ops_t = wp.tile([O, S], bf16)
            idn64 = wp.tile([64, 64], bf16)
            idn128 = wp.tile([128, 128], bf16)
            bvb = wp.tile([16, H], bf16)
            pen = wp.tile([128, 4, H], bf16)

            def dma2(t, d):
                half = t.shape[1] // 2
                nc.scalar.dma_start(t[:, :half], d[:, :half])
                nc.scalar.dma_start(t[:, half:], d[:, half:])

            # embedding-critical loads first (all on the scalar HWDGE queue;
            # the sync queue is kept clear for the xbar transposes)
            idxs = []
            for b in range(BPC):
                idxt = wp.tile([128, S // 16], i16, name=f"idx{b}")
                nc.scalar.dma_start(idxt[:], tok_d[b])
                idxs.append(idxt)
            dma2(pen, pen_d)
            for t, d in [(oqkt, oqkt_d), (c_sb, c_d), (ops_t, ops_d),
                         (idn64, i64b_d),
                         (idn128, i128b_d), (bvb, bvb_d)]:
                nc.scalar.dma_start(t[:], d[:])

            # ---- residual streams (bf16, both orientations)
            xt = [xp.tile([128, 8, S], bf16, name=f"xt{b}") for b in range(BPC)]
            xn = [xp.tile([128, 4, H], bf16, name=f"xn{b}") for b in range(BPC)]

            def transpose_xt_to_xn(b):
                # xn[p, c, k*128+j] = xt[j, k, c*128+p]   (verified mapping)
                for k in range(8):
                    nc.sync.dma_start(xn[b][:, :, k * 128:(k + 1) * 128],
                                      xt[b][:, k, :], transpose=True)

            def transpose_xn_to_xt(b):
                # xt[p, k, c*128+j] = xn[j, c, k*128+p]  (same xbar rule)
                for cc in range(4):
                    nc.sync.dma_start(xt[b][:, :, cc * 128:(cc + 1) * 128],
                                      xn[b][:, cc, :], transpose=True)

            # ---- embedding: row gather -> xn layout; pe add; transpose to xt
            for b in range(BPC):
                for hh in range(2):
                    nc.gpsimd.dma_gather(
                        out_ap=xn[b][:, hh * 2:(hh + 1) * 2, :],
                        in_ap=emb_d[:],
                        idxs_ap=idxs[b][:, hh * 16:(hh + 1) * 16],
                        num_idxs=S // 2, num_idxs_reg=S // 2, elem_size=H,
                        transpose=False)
            for b in range(BPC):
                for cc in range(4):
                    nc.vector.tensor_tensor(xn[b][:, cc, :], xn[b][:, cc, :],
                                            pen[:, cc, :], op=Alu.add)
            # embedding transposes on PE (identity matmuls) + PSUM copies:
            # keeps 4MB of SBUF<->SBUF xbar traffic off the DMA engines,
            # which are saturated by the gathers + weight loads at startup.
            for b in range(BPC):
                for k in range(8):
                    ttp = psS.tile([128, 512], bf16, tag="tr", bufs=2,
                                   name=f"ept_{b}_{k}")
                    for cc in range(4):
                        nc.tensor.transpose(
                            ttp[:, cc * 128:(cc + 1) * 128],
                            xn[b][:, cc, k * 128:(k + 1) * 128], idn128[:])
                    if k % 2 == 0:
                        nc.vector.tensor_copy(xt[b][:, k, :], ttp[:])
                    else:
                        nc.scalar.copy(xt[b][:, k, :], ttp[:])

            dma2(wvt, wvt_d)

            # ---- layers (stage-major emission)
            for l in range(L):
                if l == 1:
                    dma2(wot, wot_d)
                tTall = wk.tile([128, 8, 64], bf16, tag="tTall", name=f"tT_{l}")

                # S1: logits^T + shared exp (+c via ACT bias), row-sum
                es, rss = [], []
                for b in range(BPC):
                    lg = psS.tile([O, S], f32, tag="sm", name=f"lg_{l}_{b}")
                    for k in range(8):
                        nc.tensor.matmul(lg[:], oqkt[:, k, :], xt[b][:, k, :],
                                         start=(k == 0), stop=(k == 7))
                    e = sm.tile([O, S], bf16, tag="e", bufs=4, name=f"e_{l}_{b}")
                    rs = sm.tile([O, 1], f32, tag="rs", bufs=4)
                    nc.scalar.activation(e[:], lg[:], Act.Exp, bias=c_sb[:],
                                         accum_out=rs[:])
                    es.append(e)
                    rss.append(rs)

                # S2: relw path: w = (e/rs) * ops, ws = row-sum(w)
                wbs, wss = [], []
                for b in range(BPC):
                    rcs = sm.tile([O, 1], f32, tag="rcs", bufs=4)
                    nc.vector.reciprocal(rcs[:], rss[b][:])
                    wb = sm.tile([O, S], bf16, tag="wb", bufs=4,
                                 name=f"wb_{l}_{b}")
                    ws = sm.tile([O, 1], f32, tag="wsc", bufs=4)
                    nc.vector.scalar_tensor_tensor(wb[:], es[b][:], rcs[:],
                                                   ops_t[:], op0=Alu.mult,
                                                   op1=Alu.mult,
                                                   accum_out=ws[:])
                    wbs.append(wb)
                    wss.append(ws)

                # S2b: opw path: column-softmax of e (Pool all-reduce + divide)
                opwts = []
                for b in range(BPC):
                    csb = sm.tile([O, S], f32, tag="csb", bufs=4,
                                  name=f"csb_{l}_{b}")
                    nc.gpsimd.partition_all_reduce(
                        csb[:], es[b][:], channels=O,
                        reduce_op=bass_isa.ReduceOp.add)
                    rcb = sm.tile([O, S], f32, tag="rcb", bufs=4,
                                  name=f"rcb_{l}_{b}")
                    nc.vector.reciprocal(rcb[:], csb[:])
                    opwt = sm.tile([O, S], bf16, tag="opwt", bufs=6,
                                   name=f"opwt_{l}_{b}")
                    nc.vector.tensor_tensor(opwt[:], es[b][:], rcb[:],
                                            op=Alu.mult)
                    opwts.append(opwt)

                # S3: w^T chunks (PE transposes) + ws row transpose
                wts = []
                for b in range(BPC):
                    wtp = psS.tile([128, 4, O], bf16, tag="tr", bufs=2)
                    for cc in range(4):
                        nc.tensor.transpose(wtp[:, cc, :],
                                            wbs[b][:, cc * 128:(cc + 1) * 128],
                                            idn64[0:16, 0:16])
                    wt = sm.tile([128, 4, O], bf16, tag="wt", bufs=4,
                                 name=f"wt_{l}_{b}")
                    nc.vector.tensor_copy(wt[:], wtp[:])
                    wts.append(wt)

                # S4-S6 in HALF-BATCH groups: t, t^T, op_out (M=32),
                # select+bias — group 0 flows into outT/resid while group 1
                # is still in its t phase (shorter layer critical path).
                oos = []
                for gg in range(2):
                    gbs = [2 * gg, 2 * gg + 1]
                    tbs = {}
                    for b in gbs:
                        tb = sm.tile([O, H], bf16, tag="tb", bufs=4,
                                     name=f"tb_{l}_{b}")
                        tp0 = psA.tile([O, 512], f32, tag="big",
                                       name=f"t_{l}_{b}_0")
                        tp1 = psA.tile([O, 512], f32, tag="big",
                                       name=f"t_{l}_{b}_1")
                        for cc in range(4):
                            nc.tensor.matmul(tp0[:], wts[b][:, cc, :],
                                             xn[b][:, cc, 0:512],
                                             start=(cc == 0), stop=(cc == 3))
                            nc.tensor.matmul(tp1[:], wts[b][:, cc, :],
                                             xn[b][:, cc, 512:1024],
                                             start=(cc == 0), stop=(cc == 3))
                        nc.scalar.copy(tb[:, :512], tp0[:])
                        nc.vector.tensor_copy(tb[:, 512:], tp1[:])
                        tbs[b] = tb
                    for b in gbs:
                        for g in range(2):
                            ttp = psS.tile([128, 4, O], bf16, tag="tr", bufs=2)
                            for mm in range(4):
                                nc.tensor.transpose(
                                    ttp[:, mm, :],
                                    tbs[b][:, (g * 4 + mm) * 128:
                                           (g * 4 + mm + 1) * 128],
                                    idn64[0:16, 0:16])
                            nc.vector.tensor_copy(
                                tTall[:, g * 4:(g + 1) * 4,
                                      b * 16:(b + 1) * 16],
                                ttp[:])
                    oo_sb = sm.tile([32, H], bf16, tag="oosb", bufs=3,
                                    name=f"oo_{l}_{gg}")
                    for n in range(2):
                        oop = psO.tile([32, 512], f32, tag="oo",
                                       name=f"oop_{l}_{gg}_{n}")
                        for k in range(8):
                            nc.tensor.matmul(
                                oop[:], tTall[:, k, gg * 32:(gg + 1) * 32],
                                wvt[:, k, n * 512:(n + 1) * 512],
                                start=(k == 0), stop=(k == 7))
                        if n == 0:
                            nc.vector.tensor_copy(oo_sb[:, :512], oop[:])
                        else:
                            nc.scalar.copy(oo_sb[:, 512:], oop[:])
                    for b in gbs:
                        oob = sm.tile([O, H], bf16, tag="oob", bufs=6,
                                      name=f"oob_{l}_{b}")
                        for n in range(2):
                            sp = psS.tile([O, 512], f32, tag="tr", bufs=2,
                                          name=f"sel_{l}_{b}_{n}")
                            nc.tensor.matmul(
                                sp[:],
                                idn64[0:32, (b % 2) * 16:(b % 2) * 16 + 16],
                                oo_sb[:, n * 512:(n + 1) * 512],
                                start=True, stop=True)
                            # oob = bv * ws + sel(oo)   (bias fold, off PE)
                            nc.vector.scalar_tensor_tensor(
                                oob[:, n * 512:(n + 1) * 512],
                                bvb[:, n * 512:(n + 1) * 512], wss[b][:],
                                sp[:], op0=Alu.mult, op1=Alu.add)
                        oos.append(oob)

                # S7: out^T + residual (b-major so b0's xt frees earliest);
                # on the last layer, each batch's final projection is emitted
                # immediately after its residual so it doesn't queue behind
                # the other batches' layer-3 work in the PE stream.
                def emit_final(b):
                    # out[s,:] = x @ Wout^T  (bout added on host)
                    for mg in range(4):
                        osb = wk.tile([128, OUT], bf16, tag="osb", bufs=3)
                        pA = psA.tile([128, 512], f32, tag="big")
                        pB = psA.tile([128, 512], f32, tag="big")
                        for k in range(8):
                            nc.tensor.matmul(
                                pA[:], xt[b][:, k, mg * 128:(mg + 1) * 128],
                                wot[:, k, 0:512],
                                start=(k == 0), stop=(k == 7))
                            nc.tensor.matmul(
                                pB[:, 0:OUT - 512],
                                xt[b][:, k, mg * 128:(mg + 1) * 128],
                                wot[:, k, 512:OUT],
                                start=(k == 0), stop=(k == 7))
                        nc.vector.tensor_copy(osb[:, 0:512], pA[:])
                        nc.scalar.copy(osb[:, 512:OUT], pB[:, 0:OUT - 512])
                        nc.sync.dma_start(out_d[b, mg], osb[:])

                for b in range(BPC):
                    for m in range(8):
                        ps = psA.tile([128, 512], f32, tag="big")
                        if m % 2 == 0:
                            nc.tensor.matmul(ps[:],
                                             oos[b][:, m * 128:(m + 1) * 128],
                                             opwts[b][:], start=True, stop=True)
                            nc.vector.tensor_tensor(xt[b][:, m, :],
                                                    xt[b][:, m, :],
                                                    ps[:], op=Alu.add)
                        else:
                            nc.tensor.matmul(ps[:], idn128[:], xt[b][:, m, :],
                                             start=True, stop=False)
                            nc.tensor.matmul(ps[:],
                                             oos[b][:, m * 128:(m + 1) * 128],
                                             opwts[b][:], start=False, stop=True)
                            nc.scalar.copy(xt[b][:, m, :], ps[:])
                    if l < L - 1:
                        transpose_xt_to_xn(b)
                    else:
                        emit_final(b)

    nc.compile()
    return nc


def _prep_inputs(inputs):
    """Host-side sharding + weight-only preprocessing. in_maps for 8 cores."""
    tokens = np.asarray(inputs["tokens"]).astype(np.int64)
    word_emb = np.ascontiguousarray(np.asarray(inputs["word_emb"], dtype=np.float32))
    Wv = np.asarray(inputs["Wv"], np.float32)
    bv = np.asarray(inputs["bv"], np.float32)
    Wk = np.asarray(inputs["Wk"], np.float32)
    bk = np.asarray(inputs["bk"], np.float32)
    Wq = np.asarray(inputs["Wq_op"], np.float32)
    bq = np.asarray(inputs["bq_op"], np.float32)
    ops = np.asarray(inputs["operators"], np.float32)
    Wout = np.asarray(inputs["Wout"], np.float32)

    scale = 1.0 / math.sqrt(H)
    oq = ops @ Wq.T + bq                      # [O, H]
    oqkT = (Wk.T @ oq.T) * scale              # [H, O]
    c = (bk @ oq.T) * scale                   # [O]

    def chunked(a, nchunk):
        D, N = a.shape
        return np.ascontiguousarray(a.reshape(nchunk, 128, N).transpose(1, 0, 2))

    pe = _sinusoidal_pos_emb(S, H)            # [S, H]

    common = {
        "emb": word_emb.astype(BF16),
        "pen": np.ascontiguousarray(
            pe.reshape(4, 128, H).transpose(1, 0, 2)).astype(BF16),
        "oqkt": chunked(oqkT, 8).astype(BF16),
        "c": c.reshape(O, 1).astype(np.float32),
        "wvt": chunked(Wv.T.copy(), 8).astype(BF16),
        "wot": chunked(Wout.T.copy(), 8).astype(BF16),
        "ops": ops.astype(BF16),
        "idn64": np.eye(64, dtype=BF16),
        "idn128": np.eye(128, dtype=BF16),
        "bvb": np.tile(bv.reshape(1, H), (16, 1)).astype(BF16),
    }

    in_maps = []
    for cid in range(NCORES):
        toks = tokens[cid * BPC:(cid + 1) * BPC]
        idx = np.zeros((BPC, 128, S // 16), np.int16)
        for b in range(BPC):
            t16 = toks[b].reshape(S // 16, 16).T.astype(np.int16)
            idx[b] = np.tile(t16, (8, 1))
        in_maps.append({**common, "tok": idx})
    return in_maps


def kernel(**inputs):
    from concourse.bass_utils import run_bass_kernel_spmd

    if "nc" not in _cache:
        _cache["nc"] = _build_program()
    nc = _cache["nc"]

    in_maps = _prep_inputs(inputs)
    res = run_bass_kernel_spmd(nc, in_maps, list(range(NCORES)))
    outs = []
    for cid in range(NCORES):
        o = res.results[cid]["out"]  # [BPC, 4, 128, OUT] bf16
        outs.append(np.asarray(o, dtype=np.float32).reshape(BPC, S, OUT))
    bout = np.asarray(inputs["bout"], np.float32).reshape(1, 1, OUT)
    return np.concatenate(outs, axis=0) + bout



# revision 8
# speedup vs baseline: 1.4330x; 1.4330x over previous
"""Trainium2 Bass kernel for nn_ComposerModule (dense_transformer), v3.

Data-parallel over batch: 32 batch items -> 8 NeuronCores, 4 per core.

v3 restructure vs v2: the four per-core batch items are processed TOGETHER
in 32-partition strips of [128, S] tiles (batch b owns partitions
32b..32b+15; rows 32b+16..32b+31 are zero pads).  This turns the per-batch
[O=16, S] softmax/normalize ops into single [128, S] ops and lets the
thin-M matmuls run 4-way concurrent via tile_position row/col tiling:

  logits   lg[strip, s]     : col-tiled (0, 32b), M=32, K=128 x 8 chunks
  t = w@x  t[strip, h]      : col-tiled (0, 32b), M=32
  oo       oo[strip, h]     : shared-rhs batched, M=128
  out+res  out[s-chunk, h]  : row-tiled (32b, 0), K=32 single matmuls

The column-softmax (opw) denominator is ONE matmul with a block-diagonal
ones matrix (cs[q,s] = sum_{p in strip(q)} e[p,s]) instead of a GPSIMD
partition_all_reduce per batch, and its reciprocal runs on the ACT table
(Act.Reciprocal, ~0.6us) instead of DVE's iterative divide (~3.4us each).
w^T / t^T are HWDGE xbar DMA-transposes (scalar queue) instead of PE
identity matmuls.  All small weights ship in ONE packed dram tensor.
Residual adds are spread over DVE (tensor_tensor), ACT+GPSIMD
(copy + sbuf add), and PE+ACT (identity-accumulate + copy).

Algebra (as v2): v-projection folded, both softmaxes share one exp:
  w[o,s]  = e[o,s]/rowsum * ops[o,s];  t = w @ x;  oo = t @ Wv^T + ws*bv
  out[s,h] = sum_o e[o,s]/colsum[s] * oo[o,h];  x += out
Pad-row hygiene: oqkt pad cols are 0 and c pad rows are -30, so
e_pad = exp(-30) ~ 1e-13; ops_strip pad rows are 0 so w/ws/t/oo pads are
exactly 0 and pads contribute nothing to any contraction.
"""
import math

import numpy as np
import ml_dtypes

B, S, H, O, V, OUT, L = 32, 512, 1024, 16, 32000, 1000, 4
NCORES = 8
BPC = B // NCORES
BF16 = ml_dtypes.bfloat16

# packed-weights column offsets (bf16 [128, WC])
_PEN0 = 0              # pe chunked [128, 4*1024]
_BVB0 = 4096           # bv tiled   [128, 1024]
_OPS0 = 5120           # ops strips [128, 512]
_OQK0 = 5632           # oqkT pad   [128, 8*32]
_IDN0 = 5888           # identity   [128, 128]
_BD0 = 6016            # block-diag [128, 128]
WC = 6144

_cache = {}


def _sinusoidal_pos_emb(seq_len, dim):
    pos = np.arange(seq_len)[:, None].astype(np.float32)
    div = np.exp(np.arange(0, dim, 2).astype(np.float32) * (-math.log(10000.0) / dim))
    pe = np.zeros((seq_len, dim), dtype=np.float32)
    pe[:, 0::2] = np.sin(pos * div)
    pe[:, 1::2] = np.cos(pos * div)
    return pe


def _build_program():
    import concourse.bacc as bacc
    import concourse.bass as bass
    import concourse.tile as tile
    from concourse import mybir

    dt = mybir.dt
    f32, bf16, i16 = dt.float32, dt.bfloat16, dt.int16
    PSUM = bass.MemorySpace.PSUM
    Alu = mybir.AluOpType
    Act = mybir.ActivationFunctionType

    nc = bacc.Bacc("TRN2", target_bir_lowering=False, debug=False, num_devices=NCORES)

    emb_d = nc.declare_dram_parameter("emb", [V, H], bf16, isOutput=False)
    tok_d = nc.declare_dram_parameter("tok", [128, BPC, S // 16], i16, isOutput=False)
    wpk_d = nc.declare_dram_parameter("wpk", [128, WC], bf16, isOutput=False)
    cst_d = nc.declare_dram_parameter("cst", [128, 1], f32, isOutput=False)
    wvt_d = nc.declare_dram_parameter("wvt", [128, 8, H], bf16, isOutput=False)
    wot_d = nc.declare_dram_parameter("wot", [128, 8, OUT], bf16, isOutput=False)
    out_d = nc.declare_dram_parameter("out", [BPC, 4, 128, OUT], bf16, isOutput=True)

    with tile.TileContext(nc) as tc:
        with (
            tc.tile_pool(name="wts", bufs=1) as wp,
            tc.tile_pool(name="xres", bufs=1) as xp,
            tc.tile_pool(name="work", bufs=2) as wk,
            tc.tile_pool(name="sm", bufs=2) as sm,
            tc.tile_pool(name="psL", bufs=1, space=PSUM) as psL,
            tc.tile_pool(name="psW", bufs=2, space=PSUM) as psW,
            tc.tile_pool(name="psO", bufs=5, space=PSUM) as psO,
        ):
            # ---- persistent weights
            wpk = wp.tile([128, WC], bf16)
            c_sb = wp.tile([128, 1], f32)
            wvt = wp.tile([128, 8, H], bf16)
            wot = wp.tile([128, 8, OUT], bf16)
            tokt = wp.tile([128, BPC, S // 16], i16)

            def pen(cc):
                return wpk[:, _PEN0 + cc * H:_PEN0 + (cc + 1) * H]

            def bvb(n):
                return wpk[:, _BVB0 + n * 512:_BVB0 + (n + 1) * 512]

            ops_s = wpk[:, _OPS0:_OPS0 + 512]

            def oqkt(k):
                return wpk[:, _OQK0 + k * 32:_OQK0 + (k + 1) * 32]

            idn = wpk[:, _IDN0:_IDN0 + 128]
            bd = wpk[:, _BD0:_BD0 + 128]

            # startup loads: tok + packed weights on sync, wvt/wot on scalar
            nc.sync.dma_start(tokt[:], tok_d[:])
            nc.sync.dma_start(wpk[:], wpk_d[:])
            nc.sync.dma_start(c_sb[:], cst_d[:])
            nc.scalar.dma_start(wvt[:, 0:4], wvt_d[:, 0:4])
            nc.scalar.dma_start(wvt[:, 4:8], wvt_d[:, 4:8])

            # ---- residual streams (bf16, both orientations)
            xt = [xp.tile([128, 8, S], bf16, name=f"xt{b}") for b in range(BPC)]
            xn = [xp.tile([128, 4, H], bf16, name=f"xn{b}") for b in range(BPC)]

            # ---- embedding: SWDGE row gather -> xn layout; +pe; xbar -> xt0
            for b in range(BPC):
                for hh in range(2):
                    nc.gpsimd.dma_gather(
                        out_ap=xn[b][:, hh * 2:(hh + 1) * 2, :],
                        in_ap=emb_d[:],
                        idxs_ap=tokt[:, b, hh * 16:(hh + 1) * 16],
                        num_idxs=S // 2, num_idxs_reg=S // 2, elem_size=H,
                        transpose=False)
            for b in range(BPC):
                for cc in range(4):
                    nc.vector.tensor_tensor(xn[b][:, cc, :], xn[b][:, cc, :],
                                            pen(cc), op=Alu.add)
                for cc in range(4):
                    nc.sync.dma_start(xt[b][:, :, cc * 128:(cc + 1) * 128],
                                      xn[b][:, cc, :], transpose=True)

            # final-projection weights (needed only after the last layer)
            nc.scalar.dma_start(wot[:, 0:4], wot_d[:, 0:4])
            nc.scalar.dma_start(wot[:, 4:8], wot_d[:, 4:8])

            def emit_final(b):
                # out[s, :] = x @ Wout^T  (bout added on host)
                for mg in range(4):
                    osb = wk.tile([128, OUT], bf16, tag="osb", bufs=3)
                    pA = psO.tile([128, 512], f32, tag="rs", name=f"fA_{b}_{mg}")
                    pB = psO.tile([128, 512], f32, tag="rs", name=f"fB_{b}_{mg}")
                    for k in range(8):
                        nc.tensor.matmul(
                            pA[:], xt[b][:, k, mg * 128:(mg + 1) * 128],
                            wot[:, k, 0:512], start=(k == 0), stop=(k == 7))
                        nc.tensor.matmul(
                            pB[:, 0:OUT - 512],
                            xt[b][:, k, mg * 128:(mg + 1) * 128],
                            wot[:, k, 512:OUT], start=(k == 0), stop=(k == 7))
                    nc.vector.tensor_copy(osb[:, 0:512], pA[:])
                    nc.scalar.copy(osb[:, 512:OUT], pB[:, 0:OUT - 512])
                    nc.sync.dma_start(out_d[b, mg], osb[:])

            # ---- layers (stage-major emission, all 4 batches per stage)
            for l in range(L):
                # S1: logits, col-tiled 4-way: lg[32b.., s]
                lg = psL.tile([128, S], f32, tag="lg", name=f"lg_{l}")
                for k in range(8):
                    for b in range(BPC):
                        nc.tensor.matmul(lg[32 * b:32 * (b + 1), :],
                                         oqkt(k), xt[b][:, k, :],
                                         start=(k == 0), stop=(k == 7),
                                         tile_position=(0, 32 * b))

                # S2: shared exp (+c via ACT bias), row-sum for relw denom
                e_all = sm.tile([128, S], bf16, tag="e", bufs=2, name=f"e_{l}")
                rs = sm.tile([128, 1], f32, tag="rs", bufs=2)
                nc.scalar.activation(e_all[:], lg[:], Act.Exp, bias=c_sb[:],
                                     accum_out=rs[:])

                # S3a: colsum via block-diag matmul; ACT-table reciprocal
                cs_ps = psL.tile([128, S], f32, tag="lg", name=f"cs_{l}")
                nc.tensor.matmul(cs_ps[:], bd[:], e_all[:], start=True,
                                 stop=True)
                rcb = sm.tile([128, S], f32, tag="rcb", bufs=2,
                              name=f"rcb_{l}")
                nc.vector.reciprocal_approx_fast(rcb[:], cs_ps[:])

                # S3b: relw path: w = (e*rcs)*ops, ws = row-sum(w)
                rcs = sm.tile([128, 1], f32, tag="rcs", bufs=2)
                nc.vector.reciprocal(rcs[:], rs[:])
                w_all = sm.tile([128, S], bf16, tag="w", bufs=2, name=f"w_{l}")
                ws = sm.tile([128, 1], f32, tag="ws", bufs=2)
                nc.vector.scalar_tensor_tensor(w_all[:], e_all[:], rcs[:],
                                               ops_s, op0=Alu.mult,
                                               op1=Alu.mult, accum_out=ws[:])

                # S3c: opw numerator: opwt = e * (1/colsum broadcast)
                opwt = sm.tile([128, S], bf16, tag="opwt", bufs=2,
                               name=f"opwt_{l}")
                nc.vector.tensor_tensor(opwt[:], e_all[:], rcb[:], op=Alu.mult)

                # S4: w^T via xbar (scalar queue), then t (col-tiled 4-way)
                wt_sb = sm.tile([128, 4, 128], bf16, tag="wt", bufs=2,
                                name=f"wt_{l}")
                nc.scalar.dma_start(wt_sb[:], w_all[:], transpose=True)
                t_ps = [psW.tile([128, 512], f32, tag="tw",
                                 name=f"t_{l}_{n}") for n in range(2)]
                for cc in range(4):
                    for n in range(2):
                        for b in range(BPC):
                            nc.tensor.matmul(
                                t_ps[n][32 * b:32 * (b + 1), :],
                                wt_sb[:, cc, 32 * b:32 * (b + 1)],
                                xn[b][:, cc, n * 512:(n + 1) * 512],
                                start=(cc == 0), stop=(cc == 3),
                                tile_position=(0, 32 * b))
                t_sb = sm.tile([128, H], bf16, tag="tsb", bufs=2,
                               name=f"t_{l}")
                nc.scalar.copy(t_sb[:, 0:512], t_ps[0][:])
                nc.scalar.copy(t_sb[:, 512:], t_ps[1][:])

                # S5: t^T via xbar, then oo (shared-rhs batched, M=128)
                tt_sb = sm.tile([128, 8, 128], bf16, tag="tt", bufs=2,
                                name=f"tt_{l}")
                nc.scalar.dma_start(tt_sb[:, 0:4], t_sb[:, 0:512],
                                    transpose=True)
                nc.scalar.dma_start(tt_sb[:, 4:8], t_sb[:, 512:],
                                    transpose=True)
                oo_ps = [psW.tile([128, 512], f32, tag="tw",
                                  name=f"oo_{l}_{n}") for n in range(2)]
                for n in range(2):
                    for k in range(8):
                        nc.tensor.matmul(oo_ps[n][:], tt_sb[:, k, :],
                                         wvt[:, k, n * 512:(n + 1) * 512],
                                         start=(k == 0), stop=(k == 7))
                oo_sb = sm.tile([128, H], bf16, tag="oo", bufs=2,
                                name=f"oo_{l}")
                for n in range(2):
                    nc.vector.scalar_tensor_tensor(
                        oo_sb[:, n * 512:(n + 1) * 512], bvb(n), ws[:],
                        oo_ps[n][:], op0=Alu.mult, op1=Alu.add)

                # S6: out + residual.  n=0 half: DVE psum+sbuf add.
                # n=1 half: b 0/1 ACT copy + GPSIMD sbuf add, b 2/3 PE
                # identity-accumulate + ACT copy (engine balance).
                for cc in range(4):
                    pos = []
                    for b in range(BPC):
                        po = psO.tile([128, 512], f32, tag="rs",
                                      name=f"o_{l}_{cc}_{b}")
                        nc.tensor.matmul(
                            po[:],
                            opwt[32 * b:32 * (b + 1),
                                 cc * 128:(cc + 1) * 128],
                            oo_sb[32 * b:32 * (b + 1), 0:512],
                            start=True, stop=True,
                            tile_position=(32 * b, 0))
                        pos.append(po)
                    for b in range(BPC):
                        nc.vector.tensor_tensor(xn[b][:, cc, 0:512],
                                                xn[b][:, cc, 0:512],
                                                pos[b][:], op=Alu.add)
                    qs = []
                    for b in range(BPC):
                        q = psO.tile([128, 512], f32, tag="rs",
                                     name=f"q_{l}_{cc}_{b}")
                        if b < 2:
                            nc.tensor.matmul(
                                q[:],
                                opwt[32 * b:32 * (b + 1),
                                     cc * 128:(cc + 1) * 128],
                                oo_sb[32 * b:32 * (b + 1), 512:1024],
                                start=True, stop=True,
                                tile_position=(32 * b, 0))
                        else:
                            nc.tensor.matmul(q[:], idn,
                                             xn[b][:, cc, 512:1024],
                                             start=True, stop=False)
                            nc.tensor.matmul(
                                q[:],
                                opwt[32 * b:32 * (b + 1),
                                     cc * 128:(cc + 1) * 128],
                                oo_sb[32 * b:32 * (b + 1), 512:1024],
                                start=False, stop=True,
                                skip_group_check=True,
                                tile_position=(32 * b, 0))
                        qs.append(q)
                    for b in range(2):
                        rtmp = sm.tile([128, 512], bf16, tag="rtmp", bufs=2,
                                       name=f"rt_{l}_{cc}_{b}")
                        nc.scalar.copy(rtmp[:], qs[b][:])
                        nc.gpsimd.tensor_tensor(xn[b][:, cc, 512:1024],
                                                xn[b][:, cc, 512:1024],
                                                rtmp[:], op=Alu.add)
                    for b in range(2, BPC):
                        nc.scalar.copy(xn[b][:, cc, 512:1024], qs[b][:])

                # regenerate xt for the next layer / final projection
                for b in range(BPC):
                    for cc in range(4):
                        nc.sync.dma_start(
                            xt[b][:, :, cc * 128:(cc + 1) * 128],
                            xn[b][:, cc, :], transpose=True)
                    if l == L - 1:
                        emit_final(b)

    nc.compile()
    return nc


def _prep_inputs(inputs):
    """Host-side sharding + weight-only preprocessing. in_maps for 8 cores."""
    tokens = np.asarray(inputs["tokens"]).astype(np.int64)
    word_emb = np.ascontiguousarray(np.asarray(inputs["word_emb"], np.float32))
    Wv = np.asarray(inputs["Wv"], np.float32)
    bv = np.asarray(inputs["bv"], np.float32)
    Wk = np.asarray(inputs["Wk"], np.float32)
    bk = np.asarray(inputs["bk"], np.float32)
    Wq = np.asarray(inputs["Wq_op"], np.float32)
    bq = np.asarray(inputs["bq_op"], np.float32)
    ops = np.asarray(inputs["operators"], np.float32)
    Wout = np.asarray(inputs["Wout"], np.float32)

    scale = 1.0 / math.sqrt(H)
    oq = ops @ Wq.T + bq                      # [O, H]
    oqkT = (Wk.T @ oq.T) * scale              # [H, O]
    c = (bk @ oq.T) * scale                   # [O]

    def chunked(a):
        D, N = a.shape
        return np.ascontiguousarray(a.reshape(8, 128, N).transpose(1, 0, 2))

    pe = _sinusoidal_pos_emb(S, H)            # [S, H]
    pen = np.ascontiguousarray(
        pe.reshape(4, 128, H).transpose(1, 0, 2)).reshape(128, 4 * H)

    oqkt_pad = np.zeros((128, 8, 32), np.float32)
    oqkt_pad[:, :, :O] = chunked(oqkT)

    c_strip = np.full((128, 1), -30.0, np.float32)
    ops_strip = np.zeros((128, 512), np.float32)
    for b4 in range(4):
        c_strip[32 * b4:32 * b4 + O, 0] = c
        ops_strip[32 * b4:32 * b4 + O] = ops

    bd = np.zeros((128, 128), np.float32)
    for b4 in range(4):
        bd[32 * b4:32 * (b4 + 1), 32 * b4:32 * (b4 + 1)] = 1.0

    wpk = np.concatenate([
        pen,
        np.tile(bv.reshape(1, H), (128, 1)),
        ops_strip,
        oqkt_pad.reshape(128, 8 * 32),
        np.eye(128, dtype=np.float32),
        bd,
    ], axis=1).astype(BF16)
    assert wpk.shape == (128, WC), wpk.shape

    common = {
        "emb": word_emb.astype(BF16),
        "wpk": wpk,
        "cst": c_strip,
        "wvt": chunked(Wv.T.copy()).astype(BF16),
        "wot": chunked(Wout.T.copy()).astype(BF16),
    }

    in_maps = []
    for cid in range(NCORES):
        toks = tokens[cid * BPC:(cid + 1) * BPC]
        idx = np.zeros((128, BPC, S // 16), np.int16)
        for b in range(BPC):
            t16 = toks[b].reshape(S // 16, 16).T.astype(np.int16)
            idx[:, b, :] = np.tile(t16, (8, 1))
        in_maps.append({**common, "tok": idx})
    return in_maps


def kernel(**inputs):
    from concourse.bass_utils import run_bass_kernel_spmd

    if "nc" not in _cache:
        _cache["nc"] = _build_program()
    nc = _cache["nc"]

    in_maps = _prep_inputs(inputs)
    res = run_bass_kernel_spmd(nc, in_maps, list(range(NCORES)))
    outs = []
    for cid in range(NCORES):
        o = res.results[cid]["out"]  # [BPC, 4, 128, OUT] bf16
        outs.append(np.asarray(o, dtype=np.float32).reshape(BPC, S, OUT))
    bout = np.asarray(inputs["bout"], np.float32).reshape(1, 1, OUT)
    return np.concatenate(outs, axis=0) + bout


# revision 11
# speedup vs baseline: 1.4375x; 1.0032x over previous
"""Trainium2 Bass kernel for nn_ComposerModule (dense_transformer), v4.

Data-parallel over batch: 32 batch items -> 8 NeuronCores, 4 per core.

The four per-core batch items are processed TOGETHER in 32-partition strips
of [128, S] tiles (batch b owns partitions 32b..32b+15; rows 32b+16..32b+31
are zero pads).  Per-batch [O=16, S] softmax ops become single [128, S]
ops; thin-M matmuls run 4-way concurrent via tile_position tiling.

v4 key change vs v3: the residual stream is kept ONLY in xn ([s, h])
orientation per layer.  The logits for layer l+1 are computed
INCREMENTALLY in f32 PSUM instead of re-projecting x each layer:
    lg_{l+1} = lg_l + G @ opwt,   G = t @ (Wv^T oqk) + ws x (bv oqk)
(8 N=32 matmuls + one 32x32-tile matmul per batch).  This removes the 16
xbar xn->xt DMA-transposes per layer -- the profiled v3 bottleneck (all
DMA-transposes serialize globally against each other and against SWDGE
gathers) -- and decouples layer l+1's softmax front from layer l's
residual adds.  xt is materialized only at layer 0 (PE transposes of the
gathered embedding, which can overlap the SWDGE gathers) and after the
last layer (xbar transposes feeding the final projection).

The column-softmax denominator is ONE matmul with a block-diagonal ones
matrix; its reciprocal is DVE reciprocal_approx_fast (one Newton pass).
w^T / t^T are HWDGE xbar DMA-transposes.  Residual adds are spread over
DVE (psum add), ACT+GPSIMD (copy + sbuf add), and PE+ACT (identity-
accumulate + copy).  All small weights ship in ONE packed dram tensor.

Algebra: v-projection folded, both softmaxes share one exp:
  w[o,s]  = e[o,s]/rowsum * ops[o,s];  t = w @ x;  oo = t @ Wv^T + ws*bv
  out[s,h] = sum_o e[o,s]/colsum[s] * oo[o,h];  x += out
Pad hygiene: oqkt pad cols are 0 and c pad rows are -30, so
e_pad = exp(-30) ~ 1e-13; ops_strip/A2/g0 pad entries are 0 so
w/ws/t/oo/G/delta-lg pads are exactly 0.
"""
import math

import numpy as np
import ml_dtypes

B, S, H, O, V, OUT, L = 32, 512, 1024, 16, 32000, 1000, 4
NCORES = 8
BPC = B // NCORES
BF16 = ml_dtypes.bfloat16

# packed-weights column offsets (bf16 [128, WC])
_PEN0 = 0              # pe chunked [128, 4*1024]
_BVB0 = 4096           # bv tiled   [128, 1024]
_OPS0 = 5120           # ops strips [128, 512]
_OQK0 = 5632           # oqkT pad   [128, 8*32]
_IDN0 = 5888           # identity   [128, 128]
_BD0 = 6016            # block-diag [128, 128]
_A20 = 6144            # Wv^T@oqkT pad [128, 8*32]
_G00 = 6400            # bv@oqkT pad   [128, 32]
WC = 6432

_cache = {}


def _sinusoidal_pos_emb(seq_len, dim):
    pos = np.arange(seq_len)[:, None].astype(np.float32)
    div = np.exp(np.arange(0, dim, 2).astype(np.float32) * (-math.log(10000.0) / dim))
    pe = np.zeros((seq_len, dim), dtype=np.float32)
    pe[:, 0::2] = np.sin(pos * div)
    pe[:, 1::2] = np.cos(pos * div)
    return pe


def _build_program():
    import concourse.bacc as bacc
    import concourse.bass as bass
    import concourse.tile as tile
    from concourse import mybir

    dt = mybir.dt
    f32, bf16, i16 = dt.float32, dt.bfloat16, dt.int16
    PSUM = bass.MemorySpace.PSUM
    Alu = mybir.AluOpType
    Act = mybir.ActivationFunctionType

    nc = bacc.Bacc("TRN2", target_bir_lowering=False, debug=False, num_devices=NCORES)

    emb_d = nc.declare_dram_parameter("emb", [V, H], bf16, isOutput=False)
    tok_d = nc.declare_dram_parameter("tok", [128, BPC, S // 16], i16, isOutput=False)
    wpk_d = nc.declare_dram_parameter("wpk", [128, WC], bf16, isOutput=False)
    cst_d = nc.declare_dram_parameter("cst", [128, 1], f32, isOutput=False)
    wvt_d = nc.declare_dram_parameter("wvt", [128, 8, H], bf16, isOutput=False)
    wot_d = nc.declare_dram_parameter("wot", [128, 8, OUT], bf16, isOutput=False)
    out_d = nc.declare_dram_parameter("out", [BPC, 4, 128, OUT], bf16, isOutput=True)

    with tile.TileContext(nc) as tc:
        with (
            tc.tile_pool(name="wts", bufs=1) as wp,
            tc.tile_pool(name="xres", bufs=1) as xp,
            tc.tile_pool(name="work", bufs=2) as wk,
            tc.tile_pool(name="sm", bufs=2) as sm,
            tc.tile_pool(name="psG", bufs=1, space=PSUM) as psG,
            tc.tile_pool(name="psW", bufs=2, space=PSUM) as psW,
            tc.tile_pool(name="psO", bufs=3, space=PSUM) as psO,
            tc.tile_pool(name="psT", bufs=2, space=PSUM) as psT,
        ):
            # ---- persistent weights
            wpk = wp.tile([128, WC], bf16)
            c_sb = wp.tile([128, 1], f32)
            wvt = wp.tile([128, 8, H], bf16)
            wot = wp.tile([128, 8, OUT], bf16)
            tokt = wp.tile([128, BPC, S // 16], i16)

            def pen(cc):
                return wpk[:, _PEN0 + cc * H:_PEN0 + (cc + 1) * H]

            def bvb(n):
                return wpk[:, _BVB0 + n * 512:_BVB0 + (n + 1) * 512]

            ops_s = wpk[:, _OPS0:_OPS0 + 512]

            def oqkt(k):
                return wpk[:, _OQK0 + k * 32:_OQK0 + (k + 1) * 32]

            idn = wpk[:, _IDN0:_IDN0 + 128]
            bd = wpk[:, _BD0:_BD0 + 128]

            def a2p(k):
                return wpk[:, _A20 + k * 32:_A20 + (k + 1) * 32]

            g0b = wpk[:, _G00:_G00 + 32]

            # startup loads: tok + packed weights on sync, wvt/wot on scalar
            nc.sync.dma_start(tokt[:], tok_d[:])
            nc.sync.dma_start(wpk[:], wpk_d[:])
            nc.sync.dma_start(c_sb[:], cst_d[:])
            nc.scalar.dma_start(wvt[:, 0:4], wvt_d[:, 0:4])
            nc.scalar.dma_start(wvt[:, 4:8], wvt_d[:, 4:8])

            # ---- residual stream (bf16): xn master; xt only at entry/exit
            xt = [xp.tile([128, 8, S], bf16, name=f"xt{b}") for b in range(BPC)]
            xn = [xp.tile([128, 4, H], bf16, name=f"xn{b}") for b in range(BPC)]

            # persistent f32 logits accumulator [strip, s]
            lg = psG.tile([128, S], f32, name="lg")

            # ---- embedding: SWDGE gather -> xn; +pe; PE-transpose -> xt0;
            # layer-0 logits emitted per batch (keeps the PE FIFO flowing)
            for b in range(BPC):
                for hh in range(2):
                    nc.gpsimd.dma_gather(
                        out_ap=xn[b][:, hh * 2:(hh + 1) * 2, :],
                        in_ap=emb_d[:],
                        idxs_ap=tokt[:, b, hh * 16:(hh + 1) * 16],
                        num_idxs=S // 2, num_idxs_reg=S // 2, elem_size=H,
                        transpose=False)
            for b in range(BPC):
                for cc in range(4):
                    nc.vector.tensor_tensor(xn[b][:, cc, :], xn[b][:, cc, :],
                                            pen(cc), op=Alu.add)
                for k in range(8):
                    ttp = psT.tile([128, 4, 128], bf16, tag="tr", bufs=2,
                                   name=f"ept_{b}_{k}")
                    for cc in range(4):
                        nc.tensor.transpose(
                            ttp[:, cc, :],
                            xn[b][:, cc, k * 128:(k + 1) * 128], idn)
                    if k % 2 == 0:
                        nc.vector.tensor_copy(xt[b][:, k, :], ttp[:])
                    else:
                        nc.scalar.copy(xt[b][:, k, :], ttp[:])
                for k in range(8):
                    nc.tensor.matmul(lg[32 * b:32 * (b + 1), :],
                                     oqkt(k), xt[b][:, k, :],
                                     start=(k == 0), stop=False,
                                     tile_position=(0, 32 * b),
                                     skip_group_check=True)

            # final-projection weights (needed only after the last layer)
            nc.scalar.dma_start(wot[:, 0:4], wot_d[:, 0:4])
            nc.scalar.dma_start(wot[:, 4:8], wot_d[:, 4:8])

            def emit_final(b):
                # out[s, :] = x @ Wout^T  (bout added on host)
                for cc in range(4):
                    nc.sync.dma_start(xt[b][:, :, cc * 128:(cc + 1) * 128],
                                      xn[b][:, cc, :], transpose=True)
                for mg in range(4):
                    osb = wk.tile([128, OUT], bf16, tag="osb", bufs=3)
                    pA = psO.tile([128, 512], f32, tag="rs", name=f"fA_{b}_{mg}")
                    pB = psO.tile([128, 512], f32, tag="rs", name=f"fB_{b}_{mg}")
                    for k in range(8):
                        nc.tensor.matmul(
                            pA[:], xt[b][:, k, mg * 128:(mg + 1) * 128],
                            wot[:, k, 0:512], start=(k == 0), stop=(k == 7))
                        nc.tensor.matmul(
                            pB[:, 0:OUT - 512],
                            xt[b][:, k, mg * 128:(mg + 1) * 128],
                            wot[:, k, 512:OUT], start=(k == 0), stop=(k == 7))
                    nc.vector.tensor_copy(osb[:, 0:512], pA[:])
                    nc.scalar.copy(osb[:, 512:OUT], pB[:, 0:OUT - 512])
                    nc.sync.dma_start(out_d[b, mg], osb[:])

            # ---- layers (stage-major emission, all 4 batches per stage)
            for l in range(L):
                # S2: shared exp (+c via ACT bias), row-sum for relw denom
                e_all = sm.tile([128, S], bf16, tag="e", bufs=2, name=f"e_{l}")
                rs = sm.tile([128, 1], f32, tag="rs", bufs=2)
                nc.scalar.activation(e_all[:], lg[:], Act.Exp, bias=c_sb[:],
                                     accum_out=rs[:])

                # S3a: colsum via block-diag matmul; fast approx reciprocal
                cs_ps = psW.tile([128, S], f32, tag="tw", name=f"cs_{l}")
                nc.tensor.matmul(cs_ps[:], bd[:], e_all[:], start=True,
                                 stop=True)
                rcb = sm.tile([128, S], f32, tag="rcb", bufs=2,
                              name=f"rcb_{l}")
                nc.vector.reciprocal_approx_fast(rcb[:], cs_ps[:])

                # S3b: relw path: w = (e*rcs)*ops, ws = row-sum(w)
                rcs = sm.tile([128, 1], f32, tag="rcs", bufs=2)
                nc.vector.reciprocal(rcs[:], rs[:])
                w_all = sm.tile([128, S], bf16, tag="w", bufs=2, name=f"w_{l}")
                ws = sm.tile([128, 1], f32, tag="ws", bufs=2)
                nc.vector.scalar_tensor_tensor(w_all[:], e_all[:], rcs[:],
                                               ops_s, op0=Alu.mult,
                                               op1=Alu.mult, accum_out=ws[:])

                # S3c: opw numerator: opwt = e * (1/colsum broadcast)
                opwt = sm.tile([128, S], bf16, tag="opwt", bufs=2,
                               name=f"opwt_{l}")
                nc.vector.tensor_tensor(opwt[:], e_all[:], rcb[:], op=Alu.mult)

                # S4: w^T via xbar (sync queue), then t (col-tiled 4-way)
                wt_sb = sm.tile([128, 4, 128], bf16, tag="wt", bufs=2,
                                name=f"wt_{l}")
                nc.sync.dma_start(wt_sb[:], w_all[:], transpose=True)
                t_ps = [psW.tile([128, 512], f32, tag="tw",
                                 name=f"t_{l}_{n}") for n in range(2)]
                for cc in range(4):
                    for n in range(2):
                        for b in range(BPC):
                            nc.tensor.matmul(
                                t_ps[n][32 * b:32 * (b + 1), :],
                                wt_sb[:, cc, 32 * b:32 * (b + 1)],
                                xn[b][:, cc, n * 512:(n + 1) * 512],
                                start=(cc == 0), stop=(cc == 3),
                                tile_position=(0, 32 * b))
                t_sb = sm.tile([128, H], bf16, tag="tsb", bufs=2,
                               name=f"t_{l}")
                nc.scalar.copy(t_sb[:, 0:512], t_ps[0][:])
                nc.scalar.copy(t_sb[:, 512:], t_ps[1][:])

                # S5: t^T via xbar, then oo (shared-rhs batched, M=128)
                tt_sb = sm.tile([128, 8, 128], bf16, tag="tt", bufs=2,
                                name=f"tt_{l}")
                nc.sync.dma_start(tt_sb[:, 0:4], t_sb[:, 0:512],
                                  transpose=True)
                nc.sync.dma_start(tt_sb[:, 4:8], t_sb[:, 512:],
                                  transpose=True)
                oo_ps = [psW.tile([128, 512], f32, tag="tw",
                                  name=f"oo_{l}_{n}") for n in range(2)]
                for n in range(2):
                    for k in range(8):
                        nc.tensor.matmul(oo_ps[n][:], tt_sb[:, k, :],
                                         wvt[:, k, n * 512:(n + 1) * 512],
                                         start=(k == 0), stop=(k == 7))
                oo_sb = sm.tile([128, H], bf16, tag="oo", bufs=2,
                                name=f"oo_{l}")
                for n in range(2):
                    nc.vector.scalar_tensor_tensor(
                        oo_sb[:, n * 512:(n + 1) * 512], bvb(n), ws[:],
                        oo_ps[n][:], op0=Alu.mult, op1=Alu.add)

                # S5b: incremental logits for the next layer:
                # G = t @ A2 + ws x g0;  lg += G^T-strips @ opwt
                if l < L - 1:
                    g_full = psW.tile([128, 512], f32, tag="tw",
                                      name=f"g_{l}")
                    g_ps = g_full[:, 0:32]
                    for k in range(8):
                        nc.tensor.matmul(g_ps, tt_sb[:, k, :], a2p(k),
                                         start=(k == 0), stop=(k == 7))
                    g_sb = sm.tile([128, 32], bf16, tag="gsb", bufs=2,
                                   name=f"gsb_{l}")
                    nc.vector.scalar_tensor_tensor(g_sb[:], g0b, ws[:],
                                                   g_ps[:], op0=Alu.mult,
                                                   op1=Alu.add)
                    for b in range(BPC):
                        nc.tensor.matmul(lg[32 * b:32 * (b + 1), :],
                                         g_sb[32 * b:32 * (b + 1), :],
                                         opwt[32 * b:32 * (b + 1), :],
                                         start=False, stop=(l == L - 2),
                                         tile_position=(32 * b, 32 * b),
                                         skip_group_check=True)

                # S6: out + residual.  n=0 half: DVE psum+sbuf add.
                # n=1 half: b 0/1 ACT copy + GPSIMD sbuf add, b 2/3 PE
                # identity-accumulate + ACT copy (engine balance)
                for cc in range(4):
                    pos = []
                    for b in range(BPC):
                        po = psO.tile([128, 512], f32, tag="rs",
                                      name=f"o_{l}_{cc}_{b}")
                        nc.tensor.matmul(
                            po[:],
                            opwt[32 * b:32 * (b + 1),
                                 cc * 128:(cc + 1) * 128],
                            oo_sb[32 * b:32 * (b + 1), 0:512],
                            start=True, stop=True,
                            tile_position=(32 * b, 0))
                        pos.append(po)
                    for b in range(BPC):
                        nc.vector.tensor_tensor(xn[b][:, cc, 0:512],
                                                xn[b][:, cc, 0:512],
                                                pos[b][:], op=Alu.add)
                    qs = []
                    for b in range(BPC):
                        q = psO.tile([128, 512], f32, tag="rs",
                                     name=f"q_{l}_{cc}_{b}")
                        if b < 2:
                            nc.tensor.matmul(
                                q[:],
                                opwt[32 * b:32 * (b + 1),
                                     cc * 128:(cc + 1) * 128],
                                oo_sb[32 * b:32 * (b + 1), 512:1024],
                                start=True, stop=True,
                                tile_position=(32 * b, 0))
                        else:
                            nc.tensor.matmul(q[:], idn,
                                             xn[b][:, cc, 512:1024],
                                             start=True, stop=False)
                            nc.tensor.matmul(
                                q[:],
                                opwt[32 * b:32 * (b + 1),
                                     cc * 128:(cc + 1) * 128],
                                oo_sb[32 * b:32 * (b + 1), 512:1024],
                                start=False, stop=True,
                                skip_group_check=True,
                                tile_position=(32 * b, 0))
                        qs.append(q)
                    for b in range(2):
                        rtmp = sm.tile([128, 512], bf16, tag="rtmp", bufs=2,
                                       name=f"rt_{l}_{cc}_{b}")
                        nc.scalar.copy(rtmp[:], qs[b][:])
                        nc.gpsimd.tensor_tensor(xn[b][:, cc, 512:1024],
                                                xn[b][:, cc, 512:1024],
                                                rtmp[:], op=Alu.add)
                    for b in range(2, BPC):
                        nc.scalar.copy(xn[b][:, cc, 512:1024], qs[b][:])

                # last layer: xbar xn->xt + final projection, per batch
                if l == L - 1:
                    for b in range(BPC):
                        emit_final(b)

    nc.compile()
    return nc


def _prep_inputs(inputs):
    """Host-side sharding + weight-only preprocessing. in_maps for 8 cores."""
    tokens = np.asarray(inputs["tokens"]).astype(np.int64)
    word_emb = np.ascontiguousarray(np.asarray(inputs["word_emb"], np.float32))
    Wv = np.asarray(inputs["Wv"], np.float32)
    bv = np.asarray(inputs["bv"], np.float32)
    Wk = np.asarray(inputs["Wk"], np.float32)
    bk = np.asarray(inputs["bk"], np.float32)
    Wq = np.asarray(inputs["Wq_op"], np.float32)
    bq = np.asarray(inputs["bq_op"], np.float32)
    ops = np.asarray(inputs["operators"], np.float32)
    Wout = np.asarray(inputs["Wout"], np.float32)

    scale = 1.0 / math.sqrt(H)
    oq = ops @ Wq.T + bq                      # [O, H]
    oqkT = (Wk.T @ oq.T) * scale              # [H, O]
    c = (bk @ oq.T) * scale                   # [O]
    A2 = Wv.T @ oqkT                          # [H, O]
    g0 = bv @ oqkT                            # [O]

    def chunked(a):
        D, N = a.shape
        return np.ascontiguousarray(a.reshape(8, 128, N).transpose(1, 0, 2))

    def chunk_pad32(a16):                     # [H, O] -> [128, 8*32]
        out = np.zeros((128, 8, 32), np.float32)
        out[:, :, :O] = chunked(a16)
        return out.reshape(128, 8 * 32)

    pe = _sinusoidal_pos_emb(S, H)            # [S, H]
    pen = np.ascontiguousarray(
        pe.reshape(4, 128, H).transpose(1, 0, 2)).reshape(128, 4 * H)

    c_strip = np.full((128, 1), -30.0, np.float32)
    ops_strip = np.zeros((128, 512), np.float32)
    for b4 in range(4):
        c_strip[32 * b4:32 * b4 + O, 0] = c
        ops_strip[32 * b4:32 * b4 + O] = ops

    bd = np.zeros((128, 128), np.float32)
    for b4 in range(4):
        bd[32 * b4:32 * (b4 + 1), 32 * b4:32 * (b4 + 1)] = 1.0

    g0p = np.zeros((1, 32), np.float32)
    g0p[0, :O] = g0

    wpk = np.concatenate([
        pen,
        np.tile(bv.reshape(1, H), (128, 1)),
        ops_strip,
        chunk_pad32(oqkT),
        np.eye(128, dtype=np.float32),
        bd,
        chunk_pad32(A2),
        np.tile(g0p, (128, 1)),
    ], axis=1).astype(BF16)
    assert wpk.shape == (128, WC), wpk.shape

    common = {
        "emb": word_emb.astype(BF16),
        "wpk": wpk,
        "cst": c_strip,
        "wvt": chunked(Wv.T.copy()).astype(BF16),
        "wot": chunked(Wout.T.copy()).astype(BF16),
    }

    in_maps = []
    for cid in range(NCORES):
        toks = tokens[cid * BPC:(cid + 1) * BPC]
        idx = np.zeros((128, BPC, S // 16), np.int16)
        for b in range(BPC):
            t16 = toks[b].reshape(S // 16, 16).T.astype(np.int16)
            idx[:, b, :] = np.tile(t16, (8, 1))
        in_maps.append({**common, "tok": idx})
    return in_maps


def kernel(**inputs):
    from concourse.bass_utils import run_bass_kernel_spmd

    if "nc" not in _cache:
        _cache["nc"] = _build_program()
    nc = _cache["nc"]

    in_maps = _prep_inputs(inputs)
    res = run_bass_kernel_spmd(nc, in_maps, list(range(NCORES)))
    outs = []
    for cid in range(NCORES):
        o = res.results[cid]["out"]  # [BPC, 4, 128, OUT] bf16
        outs.append(np.asarray(o, dtype=np.float32).reshape(BPC, S, OUT))
    bout = np.asarray(inputs["bout"], np.float32).reshape(1, 1, OUT)
    return np.concatenate(outs, axis=0) + bout


# revision 13
# speedup vs baseline: 1.4664x; 1.0201x over previous
"""Trainium2 Bass kernel for nn_ComposerModule (dense_transformer), v5.

Data-parallel over batch: 32 batch items -> 8 NeuronCores, 4 per core.

The four per-core batch items are processed TOGETHER in 32-partition strips
of [128, S] tiles (batch b owns partitions 32b..32b+15; rows 32b+16..32b+31
are zero pads).  Per-batch [O=16, S] softmax ops become single [128, S]
ops; thin-M matmuls run 4-way concurrent via tile_position tiling.

The residual stream is kept ONLY in xn ([s, h]) orientation.  Logits are
accumulated INCREMENTALLY in f32 PSUM instead of re-projecting x:
    lg_{l+1} = lg_l + G @ opwt,   G = t @ (Wv^T oqk) + ws x (bv oqk)
xt is materialized only at layer 0 (PE transposes of the gathered
embedding, overlapping the SWDGE gathers) and after the last layer (xbar
transposes feeding the final projection).

v5 vs v4 (both profiled on HW): the embedding PSUM pool is scoped so the
layer loop gets 5 free banks; the out+residual stage is ONE K=32 N=1024
bf16-PSUM matmul per (batch, s-chunk) -- 4-way row-tile concurrent -- and
ONE [128,1024] bf16 2x-mode DVE add (batch 3 goes ACT copy + GPSIMD add
for engine balance); t^T runs on the PE (identity transposes) so the PE
has no idle window there (HAM stays warm).

Algebra: v-projection folded, both softmaxes share one exp:
  w[o,s]  = e[o,s]/rowsum * ops[o,s];  t = w @ x;  oo = t @ Wv^T + ws*bv
  out[s,h] = sum_o e[o,s]/colsum[s] * oo[o,h];  x += out
Pad hygiene: oqkt pad cols are 0 and c pad rows are -30, so
e_pad = exp(-30) ~ 1e-13; ops_strip/A2/g0 pad entries are 0 so
w/ws/t/oo/G/delta-lg pads are exactly 0.
"""
import math

import numpy as np
import ml_dtypes

B, S, H, O, V, OUT, L = 32, 512, 1024, 16, 32000, 1000, 4
NCORES = 8
BPC = B // NCORES
BF16 = ml_dtypes.bfloat16

# packed-weights column offsets (bf16 [128, WC])
_PEN0 = 0              # pe chunked [128, 4*1024]
_BVB0 = 4096           # bv tiled   [128, 1024]
_OPS0 = 5120           # ops strips [128, 512]
_OQK0 = 5632           # oqkT pad   [128, 8*32]
_IDN0 = 5888           # identity   [128, 128]
_BD0 = 6016            # block-diag [128, 128]
_A20 = 6144            # Wv^T@oqkT pad [128, 8*32]
_G00 = 6400            # bv@oqkT pad   [128, 32]
WC = 6432

_cache = {}


def _sinusoidal_pos_emb(seq_len, dim):
    pos = np.arange(seq_len)[:, None].astype(np.float32)
    div = np.exp(np.arange(0, dim, 2).astype(np.float32) * (-math.log(10000.0) / dim))
    pe = np.zeros((seq_len, dim), dtype=np.float32)
    pe[:, 0::2] = np.sin(pos * div)
    pe[:, 1::2] = np.cos(pos * div)
    return pe


def _build_program():
    import concourse.bacc as bacc
    import concourse.bass as bass
    import concourse.tile as tile
    from concourse import mybir

    dt = mybir.dt
    f32, bf16, i16 = dt.float32, dt.bfloat16, dt.int16
    PSUM = bass.MemorySpace.PSUM
    Alu = mybir.AluOpType
    Act = mybir.ActivationFunctionType

    nc = bacc.Bacc("TRN2", target_bir_lowering=False, debug=False, num_devices=NCORES)

    emb_d = nc.declare_dram_parameter("emb", [V, H], bf16, isOutput=False)
    tok_d = nc.declare_dram_parameter("tok", [128, BPC, S // 16], i16, isOutput=False)
    wpk_d = nc.declare_dram_parameter("wpk", [128, WC], bf16, isOutput=False)
    cst_d = nc.declare_dram_parameter("cst", [128, 1], f32, isOutput=False)
    wvt_d = nc.declare_dram_parameter("wvt", [128, 8, H], bf16, isOutput=False)
    wot_d = nc.declare_dram_parameter("wot", [128, 8, OUT], bf16, isOutput=False)
    out_d = nc.declare_dram_parameter("out", [BPC, 4, 128, OUT], bf16, isOutput=True)

    with tile.TileContext(nc) as tc:
        with (
            tc.tile_pool(name="wts", bufs=1) as wp,
            tc.tile_pool(name="xres", bufs=1) as xp,
            tc.tile_pool(name="work", bufs=2) as wk,
            tc.tile_pool(name="sm", bufs=2) as sm,
            tc.tile_pool(name="psG", bufs=1, space=PSUM) as psG,
            tc.tile_pool(name="psW", bufs=2, space=PSUM) as psW,
        ):
            # ---- persistent weights
            wpk = wp.tile([128, WC], bf16)
            c_sb = wp.tile([128, 1], f32)
            wvt = wp.tile([128, 8, H], bf16)
            wot = wp.tile([128, 8, OUT], bf16)
            tokt = wp.tile([128, BPC, S // 16], i16)

            def pen(cc):
                return wpk[:, _PEN0 + cc * H:_PEN0 + (cc + 1) * H]

            def bvb(n):
                return wpk[:, _BVB0 + n * 512:_BVB0 + (n + 1) * 512]

            ops_s = wpk[:, _OPS0:_OPS0 + 512]

            def oqkt(k):
                return wpk[:, _OQK0 + k * 32:_OQK0 + (k + 1) * 32]

            idn = wpk[:, _IDN0:_IDN0 + 128]
            bd = wpk[:, _BD0:_BD0 + 128]

            def a2p(k):
                return wpk[:, _A20 + k * 32:_A20 + (k + 1) * 32]

            g0b = wpk[:, _G00:_G00 + 32]

            # startup loads: tok + packed weights on sync, wvt/wot on scalar
            nc.sync.dma_start(tokt[:], tok_d[:])
            nc.sync.dma_start(wpk[:], wpk_d[:])
            nc.sync.dma_start(c_sb[:], cst_d[:])
            nc.scalar.dma_start(wvt[:, 0:4], wvt_d[:, 0:4])
            nc.scalar.dma_start(wvt[:, 4:8], wvt_d[:, 4:8])

            # ---- residual stream (bf16): xn master; xt only at entry/exit
            xt = [xp.tile([128, 8, S], bf16, name=f"xt{b}") for b in range(BPC)]
            xn = [xp.tile([128, 4, H], bf16, name=f"xn{b}") for b in range(BPC)]

            # persistent f32 logits accumulator [strip, s]
            lg = psG.tile([128, S], f32, name="lg")

            # ---- embedding: SWDGE gather -> xn; +pe; PE-transpose -> xt0;
            # layer-0 logits emitted per batch (keeps the PE FIFO flowing)
            with tc.tile_pool(name="psT", bufs=2, space=PSUM) as psT:
                for b in range(BPC):
                    for hh in range(2):
                        nc.gpsimd.dma_gather(
                            out_ap=xn[b][:, hh * 2:(hh + 1) * 2, :],
                            in_ap=emb_d[:],
                            idxs_ap=tokt[:, b, hh * 16:(hh + 1) * 16],
                            num_idxs=S // 2, num_idxs_reg=S // 2, elem_size=H,
                            transpose=False)
                for b in range(BPC):
                    for cc in range(4):
                        nc.vector.tensor_tensor(xn[b][:, cc, :],
                                                xn[b][:, cc, :],
                                                pen(cc), op=Alu.add)
                    for k in range(8):
                        ttp = psT.tile([128, 4, 128], bf16, tag="tr", bufs=2,
                                       name=f"ept_{b}_{k}")
                        for cc in range(4):
                            nc.tensor.transpose(
                                ttp[:, cc, :],
                                xn[b][:, cc, k * 128:(k + 1) * 128], idn)
                        if k % 2 == 0:
                            nc.vector.tensor_copy(xt[b][:, k, :], ttp[:])
                        else:
                            nc.scalar.copy(xt[b][:, k, :], ttp[:])
                    for k in range(8):
                        nc.tensor.matmul(lg[32 * b:32 * (b + 1), :],
                                         oqkt(k), xt[b][:, k, :],
                                         start=(k == 0), stop=False,
                                         tile_position=(0, 32 * b),
                                         skip_group_check=True)

            # final-projection weights (needed only after the last layer)
            nc.scalar.dma_start(wot[:, 0:4], wot_d[:, 0:4])
            nc.scalar.dma_start(wot[:, 4:8], wot_d[:, 4:8])

            with tc.tile_pool(name="psO", bufs=5, space=PSUM) as psO:

                def emit_final(b):
                    # out[s, :] = x @ Wout^T  (bout added on host)
                    for cc in range(4):
                        nc.sync.dma_start(
                            xt[b][:, :, cc * 128:(cc + 1) * 128],
                            xn[b][:, cc, :], transpose=True)
                    for mg in range(4):
                        osb = wk.tile([128, OUT], bf16, tag="osb", bufs=3)
                        pA = psO.tile([128, 512], f32, tag="fin", bufs=2,
                                      name=f"fA_{b}_{mg}")
                        pB = psO.tile([128, 512], f32, tag="fin", bufs=2,
                                      name=f"fB_{b}_{mg}")
                        for k in range(8):
                            nc.tensor.matmul(
                                pA[:], xt[b][:, k, mg * 128:(mg + 1) * 128],
                                wot[:, k, 0:512], start=(k == 0),
                                stop=(k == 7))
                            nc.tensor.matmul(
                                pB[:, 0:OUT - 512],
                                xt[b][:, k, mg * 128:(mg + 1) * 128],
                                wot[:, k, 512:OUT], start=(k == 0),
                                stop=(k == 7))
                        nc.vector.tensor_copy(osb[:, 0:512], pA[:])
                        nc.scalar.copy(osb[:, 512:OUT], pB[:, 0:OUT - 512])
                        nc.sync.dma_start(out_d[b, mg], osb[:])

                def emit_out_resid(l, b, cc):
                    # n=0 half: plain matmul, drained by a DVE psum add
                    po = psO.tile([128, 512], f32, tag="rs", bufs=3,
                                  name=f"o_{l}_{cc}_{b}")
                    nc.tensor.matmul(
                        po[:],
                        opwt_cur[32 * b:32 * (b + 1),
                                 cc * 128:(cc + 1) * 128],
                        oo_cur[32 * b:32 * (b + 1), 0:512],
                        start=True, stop=True, tile_position=(32 * b, 0))
                    # n=1 half: b 0/1 plain (ACT copy + GPSIMD add), b 2/3
                    # PE identity-accumulate (ACT copy)
                    q = psO.tile([128, 512], f32, tag="rs", bufs=3,
                                 name=f"q_{l}_{cc}_{b}")
                    if b >= 2:
                        nc.tensor.matmul(q[:], idn, xn[b][:, cc, 512:1024],
                                         start=True, stop=False)
                    nc.tensor.matmul(
                        q[:],
                        opwt_cur[32 * b:32 * (b + 1),
                                 cc * 128:(cc + 1) * 128],
                        oo_cur[32 * b:32 * (b + 1), 512:1024],
                        start=(b < 2), stop=True,
                        skip_group_check=True, tile_position=(32 * b, 0))
                    return po, q

                def emit_add(b, cc, poq, l):
                    po, q = poq
                    nc.vector.tensor_tensor(xn[b][:, cc, 0:512],
                                            xn[b][:, cc, 0:512],
                                            po[:], op=Alu.add)
                    if b < 2:
                        rtmp = sm.tile([128, 512], bf16, tag="rtmp", bufs=2,
                                       name=f"rt_{l}_{cc}_{b}")
                        nc.scalar.copy(rtmp[:], q[:])
                        nc.gpsimd.tensor_tensor(xn[b][:, cc, 512:1024],
                                                xn[b][:, cc, 512:1024],
                                                rtmp[:], op=Alu.add)
                    else:
                        nc.scalar.copy(xn[b][:, cc, 512:1024], q[:])

                # ---- layers (stage-major emission, all batches per stage)
                for l in range(L):
                    # S2: shared exp (+c bias), row-sum for relw denominator
                    e_all = sm.tile([128, S], bf16, tag="e", bufs=2,
                                    name=f"e_{l}")
                    rs = sm.tile([128, 1], f32, tag="rs", bufs=2)
                    nc.scalar.activation(e_all[:], lg[:], Act.Exp,
                                         bias=c_sb[:], accum_out=rs[:])

                    # S3a: colsum via block-diag matmul; fast reciprocal
                    cs_ps = psW.tile([128, S], f32, tag="tw", name=f"cs_{l}")
                    nc.tensor.matmul(cs_ps[:], bd[:], e_all[:], start=True,
                                     stop=True)
                    rcb = sm.tile([128, S], f32, tag="rcb", bufs=2,
                                  name=f"rcb_{l}")
                    nc.vector.reciprocal_approx_fast(rcb[:], cs_ps[:])

                    # S3b: relw path: w = (e*rcs)*ops, ws = row-sum(w)
                    rcs = sm.tile([128, 1], f32, tag="rcs", bufs=2)
                    nc.vector.reciprocal(rcs[:], rs[:])
                    w_all = sm.tile([128, S], bf16, tag="w", bufs=2,
                                    name=f"w_{l}")
                    ws = sm.tile([128, 1], f32, tag="ws", bufs=2)
                    nc.vector.scalar_tensor_tensor(w_all[:], e_all[:],
                                                   rcs[:], ops_s,
                                                   op0=Alu.mult,
                                                   op1=Alu.mult,
                                                   accum_out=ws[:])

                    # S3c: opw numerator: opwt = e * (1/colsum broadcast)
                    opwt = sm.tile([128, S], bf16, tag="opwt", bufs=2,
                                   name=f"opwt_{l}")
                    nc.vector.tensor_tensor(opwt[:], e_all[:], rcb[:],
                                            op=Alu.mult)

                    # S4: w^T via xbar, then t (col-tiled 4-way)
                    wt_sb = sm.tile([128, 4, 128], bf16, tag="wt", bufs=2,
                                    name=f"wt_{l}")
                    nc.sync.dma_start(wt_sb[:], w_all[:], transpose=True)
                    t_ps = [psW.tile([128, 512], f32, tag="tw",
                                     name=f"t_{l}_{n}") for n in range(2)]
                    for cc in range(4):
                        for n in range(2):
                            for b in range(BPC):
                                nc.tensor.matmul(
                                    t_ps[n][32 * b:32 * (b + 1), :],
                                    wt_sb[:, cc, 32 * b:32 * (b + 1)],
                                    xn[b][:, cc, n * 512:(n + 1) * 512],
                                    start=(cc == 0), stop=(cc == 3),
                                    tile_position=(0, 32 * b))
                    t_sb = sm.tile([128, H], bf16, tag="tsb", bufs=2,
                                   name=f"t_{l}")
                    nc.scalar.copy(t_sb[:, 0:512], t_ps[0][:])
                    nc.scalar.copy(t_sb[:, 512:], t_ps[1][:])

                    # S5: t^T on PE (keeps PE warm), then oo (M=128 batched)
                    tt_sb = sm.tile([128, 8, 128], bf16, tag="tt", bufs=2,
                                    name=f"tt_{l}")
                    for g in range(2):
                        trp = psW.tile([128, 4, 128], bf16, tag="tw",
                                       name=f"tr_{l}_{g}")
                        for k in range(4):
                            nc.tensor.transpose(
                                trp[:, k, :],
                                t_sb[:, (g * 4 + k) * 128:
                                     (g * 4 + k + 1) * 128], idn)
                        nc.vector.tensor_copy(tt_sb[:, g * 4:(g + 1) * 4],
                                              trp[:])
                    oo_ps = [psW.tile([128, 512], f32, tag="tw",
                                      name=f"oo_{l}_{n}") for n in range(2)]
                    for n in range(2):
                        for k in range(8):
                            nc.tensor.matmul(oo_ps[n][:], tt_sb[:, k, :],
                                             wvt[:, k,
                                                 n * 512:(n + 1) * 512],
                                             start=(k == 0), stop=(k == 7))
                    oo_sb = sm.tile([128, H], bf16, tag="oo", bufs=2,
                                    name=f"oo_{l}")
                    for n in range(2):
                        nc.vector.scalar_tensor_tensor(
                            oo_sb[:, n * 512:(n + 1) * 512], bvb(n), ws[:],
                            oo_ps[n][:], op0=Alu.mult, op1=Alu.add)

                    # S5b: incremental logits for the next layer:
                    # G = t @ A2 + ws x g0;  lg += G^T-strips @ opwt
                    if l < L - 1:
                        g_full = psW.tile([128, 512], f32, tag="tw",
                                          name=f"g_{l}")
                        g_ps = g_full[:, 0:32]
                        for k in range(8):
                            nc.tensor.matmul(g_ps, tt_sb[:, k, :], a2p(k),
                                             start=(k == 0), stop=(k == 7))
                        g_sb = sm.tile([128, 32], bf16, tag="gsb", bufs=2,
                                       name=f"gsb_{l}")
                        nc.vector.scalar_tensor_tensor(g_sb[:], g0b, ws[:],
                                                       g_ps, op0=Alu.mult,
                                                       op1=Alu.add)
                        for b in range(BPC):
                            nc.tensor.matmul(lg[32 * b:32 * (b + 1), :],
                                             g_sb[32 * b:32 * (b + 1), :],
                                             opwt[32 * b:32 * (b + 1), :],
                                             start=False, stop=(l == L - 2),
                                             tile_position=(32 * b, 32 * b),
                                             skip_group_check=True)

                    # S6: out + residual, one K=32 N=1024 bf16 matmul and
                    # one [128,1024] add per (b, cc).  Last layer goes
                    # batch-major so the final projection overlaps.
                    opwt_cur, oo_cur = opwt, oo_sb
                    if l < L - 1:
                        for cc in range(4):
                            pos = [emit_out_resid(l, b, cc)
                                   for b in range(BPC)]
                            for b in range(BPC):
                                emit_add(b, cc, pos[b], l)
                    else:
                        for b in range(BPC):
                            pos = [emit_out_resid(l, b, cc)
                                   for cc in range(4)]
                            for cc in range(4):
                                emit_add(b, cc, pos[cc], l)
                            emit_final(b)

    nc.compile()
    return nc


def _prep_inputs(inputs):
    """Host-side sharding + weight-only preprocessing. in_maps for 8 cores."""
    tokens = np.asarray(inputs["tokens"]).astype(np.int64)
    word_emb = np.ascontiguousarray(np.asarray(inputs["word_emb"], np.float32))
    Wv = np.asarray(inputs["Wv"], np.float32)
    bv = np.asarray(inputs["bv"], np.float32)
    Wk = np.asarray(inputs["Wk"], np.float32)
    bk = np.asarray(inputs["bk"], np.float32)
    Wq = np.asarray(inputs["Wq_op"], np.float32)
    bq = np.asarray(inputs["bq_op"], np.float32)
    ops = np.asarray(inputs["operators"], np.float32)
    Wout = np.asarray(inputs["Wout"], np.float32)

    scale = 1.0 / math.sqrt(H)
    oq = ops @ Wq.T + bq                      # [O, H]
    oqkT = (Wk.T @ oq.T) * scale              # [H, O]
    c = (bk @ oq.T) * scale                   # [O]
    A2 = Wv.T @ oqkT                          # [H, O]
    g0 = bv @ oqkT                            # [O]

    def chunked(a):
        D, N = a.shape
        return np.ascontiguousarray(a.reshape(8, 128, N).transpose(1, 0, 2))

    def chunk_pad32(a16):                     # [H, O] -> [128, 8*32]
        out = np.zeros((128, 8, 32), np.float32)
        out[:, :, :O] = chunked(a16)
        return out.reshape(128, 8 * 32)

    pe = _sinusoidal_pos_emb(S, H)            # [S, H]
    pen = np.ascontiguousarray(
        pe.reshape(4, 128, H).transpose(1, 0, 2)).reshape(128, 4 * H)

    c_strip = np.full((128, 1), -30.0, np.float32)
    ops_strip = np.zeros((128, 512), np.float32)
    for b4 in range(4):
        c_strip[32 * b4:32 * b4 + O, 0] = c
        ops_strip[32 * b4:32 * b4 + O] = ops

    bd = np.zeros((128, 128), np.float32)
    for b4 in range(4):
        bd[32 * b4:32 * (b4 + 1), 32 * b4:32 * (b4 + 1)] = 1.0

    g0p = np.zeros((1, 32), np.float32)
    g0p[0, :O] = g0

    wpk = np.concatenate([
        pen,
        np.tile(bv.reshape(1, H), (128, 1)),
        ops_strip,
        chunk_pad32(oqkT),
        np.eye(128, dtype=np.float32),
        bd,
        chunk_pad32(A2),
        np.tile(g0p, (128, 1)),
    ], axis=1).astype(BF16)
    assert wpk.shape == (128, WC), wpk.shape

    common = {
        "emb": word_emb.astype(BF16),
        "wpk": wpk,
        "cst": c_strip,
        "wvt": chunked(Wv.T.copy()).astype(BF16),
        "wot": chunked(Wout.T.copy()).astype(BF16),
    }

    in_maps = []
    for cid in range(NCORES):
        toks = tokens[cid * BPC:(cid + 1) * BPC]
        idx = np.zeros((128, BPC, S // 16), np.int16)
        for b in range(BPC):
            t16 = toks[b].reshape(S // 16, 16).T.astype(np.int16)
            idx[:, b, :] = np.tile(t16, (8, 1))
        in_maps.append({**common, "tok": idx})
    return in_maps


def kernel(**inputs):
    from concourse.bass_utils import run_bass_kernel_spmd

    if "nc" not in _cache:
        _cache["nc"] = _build_program()
    nc = _cache["nc"]

    in_maps = _prep_inputs(inputs)
    res = run_bass_kernel_spmd(nc, in_maps, list(range(NCORES)))
    outs = []
    for cid in range(NCORES):
        o = res.results[cid]["out"]  # [BPC, 4, 128, OUT] bf16
        outs.append(np.asarray(o, dtype=np.float32).reshape(BPC, S, OUT))
    bout = np.asarray(inputs["bout"], np.float32).reshape(1, 1, OUT)
    return np.concatenate(outs, axis=0) + bout


# revision 17
# speedup vs baseline: 1.5391x; 1.0495x over previous
"""Trainium2 Bass kernel for nn_ComposerModule (dense_transformer), v5.

Data-parallel over batch: 32 batch items -> 8 NeuronCores, 4 per core.

The four per-core batch items are processed TOGETHER in 32-partition strips
of [128, S] tiles (batch b owns partitions 32b..32b+15; rows 32b+16..32b+31
are zero pads).  Per-batch [O=16, S] softmax ops become single [128, S]
ops; thin-M matmuls run 4-way concurrent via tile_position tiling.

The residual stream is kept ONLY in xn ([s, h]) orientation.  Logits are
accumulated INCREMENTALLY in f32 PSUM instead of re-projecting x:
    lg_{l+1} = lg_l + G @ opwt,   G = t @ (Wv^T oqk) + ws x (bv oqk)
xt is materialized only at layer 0 (PE transposes of the gathered
embedding, overlapping the SWDGE gathers) and after the last layer (xbar
transposes feeding the final projection).

v5 vs v4 (both profiled on HW): the embedding PSUM pool is scoped so the
layer loop gets 5 free banks; the out+residual stage is ONE K=32 N=1024
bf16-PSUM matmul per (batch, s-chunk) -- 4-way row-tile concurrent -- and
ONE [128,1024] bf16 2x-mode DVE add (batch 3 goes ACT copy + GPSIMD add
for engine balance); t^T runs on the PE (identity transposes) so the PE
has no idle window there (HAM stays warm).

Algebra: v-projection folded, both softmaxes share one exp:
  w[o,s]  = e[o,s]/rowsum * ops[o,s];  t = w @ x;  oo = t @ Wv^T + ws*bv
  out[s,h] = sum_o e[o,s]/colsum[s] * oo[o,h];  x += out
Pad hygiene: oqkt pad cols are 0 and c pad rows are -30, so
e_pad = exp(-30) ~ 1e-13; ops_strip/A2/g0 pad entries are 0 so
w/ws/t/oo/G/delta-lg pads are exactly 0.
"""
import math

import numpy as np
import ml_dtypes

B, S, H, O, V, OUT, L = 32, 512, 1024, 16, 32000, 1000, 4
NCORES = 8
BPC = B // NCORES
BF16 = ml_dtypes.bfloat16

# packed-weights column offsets (bf16 [128, WC])
_PEN0 = 0              # pe chunked [128, 4*1024]
_BVB0 = 4096           # bv tiled   [128, 1024]
_OPS0 = 5120           # ops strips [128, 512]
_OQK0 = 5632           # oqkT pad   [128, 8*32]
_IDN0 = 5888           # identity   [128, 128]
_BD0 = 6016            # block-diag [128, 128]
_A20 = 6144            # Wv^T@oqkT pad [128, 8*32]
_G00 = 6400            # bv@oqkT pad   [128, 32]
WC = 6432

_cache = {}


def _sinusoidal_pos_emb(seq_len, dim):
    pos = np.arange(seq_len)[:, None].astype(np.float32)
    div = np.exp(np.arange(0, dim, 2).astype(np.float32) * (-math.log(10000.0) / dim))
    pe = np.zeros((seq_len, dim), dtype=np.float32)
    pe[:, 0::2] = np.sin(pos * div)
    pe[:, 1::2] = np.cos(pos * div)
    return pe


def _build_program():
    import concourse.bacc as bacc
    import concourse.bass as bass
    import concourse.tile as tile
    from concourse import mybir

    dt = mybir.dt
    f32, bf16, i16 = dt.float32, dt.bfloat16, dt.int16
    PSUM = bass.MemorySpace.PSUM
    Alu = mybir.AluOpType
    Act = mybir.ActivationFunctionType

    nc = bacc.Bacc("TRN2", target_bir_lowering=False, debug=False, num_devices=NCORES)

    emb_d = nc.declare_dram_parameter("emb", [V, H], bf16, isOutput=False)
    tok_d = nc.declare_dram_parameter("tok", [128, BPC, S // 16], i16, isOutput=False)
    wpk_d = nc.declare_dram_parameter("wpk", [128, WC], bf16, isOutput=False)
    cst_d = nc.declare_dram_parameter("cst", [128, 1], f32, isOutput=False)
    wvt_d = nc.declare_dram_parameter("wvt", [128, 8, H], bf16, isOutput=False)
    wot_d = nc.declare_dram_parameter("wot", [128, 8, OUT], bf16, isOutput=False)
    out_d = nc.declare_dram_parameter("out", [BPC, 4, 128, OUT], bf16, isOutput=True)

    with tile.TileContext(nc) as tc:
        with (
            tc.tile_pool(name="wts", bufs=1) as wp,
            tc.tile_pool(name="xres", bufs=1) as xp,
            tc.tile_pool(name="work", bufs=2) as wk,
            tc.tile_pool(name="sm", bufs=2) as sm,
            tc.tile_pool(name="psG", bufs=1, space=PSUM) as psG,
            tc.tile_pool(name="psW", bufs=2, space=PSUM) as psW,
        ):
            # ---- persistent weights
            wpk = wp.tile([128, WC], bf16)
            c_sb = wp.tile([128, 1], f32)
            wvt = wp.tile([128, 8, H], bf16)
            wot = wp.tile([128, 8, OUT], bf16)
            tokt = wp.tile([128, BPC, S // 16], i16)

            def pen(cc):
                return wpk[:, _PEN0 + cc * H:_PEN0 + (cc + 1) * H]

            def bvb(n):
                return wpk[:, _BVB0 + n * 512:_BVB0 + (n + 1) * 512]

            ops_s = wpk[:, _OPS0:_OPS0 + 512]

            def oqkt(k):
                return wpk[:, _OQK0 + k * 32:_OQK0 + (k + 1) * 32]

            idn = wpk[:, _IDN0:_IDN0 + 128]
            bd = wpk[:, _BD0:_BD0 + 128]

            def a2p(k):
                return wpk[:, _A20 + k * 32:_A20 + (k + 1) * 32]

            g0b = wpk[:, _G00:_G00 + 32]

            # startup loads: tok + packed weights on sync, wvt/wot on scalar
            nc.sync.dma_start(tokt[:], tok_d[:])
            nc.sync.dma_start(wpk[:], wpk_d[:])
            nc.sync.dma_start(c_sb[:], cst_d[:])
            nc.scalar.dma_start(wvt[:, 0:4], wvt_d[:, 0:4])
            nc.scalar.dma_start(wvt[:, 4:8], wvt_d[:, 4:8])

            # ---- residual stream (bf16): xn master; xt only at entry/exit
            xt = [xp.tile([128, 8, S], bf16, name=f"xt{b}") for b in range(BPC)]
            xn = [xp.tile([128, 4, H], bf16, name=f"xn{b}") for b in range(BPC)]

            # persistent f32 logits accumulator [strip, s]
            lg = psG.tile([128, S], f32, name="lg")

            # ---- embedding: SWDGE gather -> xn; +pe; PE-transpose -> xt0;
            # layer-0 logits emitted per batch (keeps the PE FIFO flowing)
            with tc.tile_pool(name="psT", bufs=2, space=PSUM) as psT:
                for b in range(BPC):
                    for hh in range(2):
                        nc.gpsimd.dma_gather(
                            out_ap=xn[b][:, hh * 2:(hh + 1) * 2, :],
                            in_ap=emb_d[:],
                            idxs_ap=tokt[:, b, hh * 16:(hh + 1) * 16],
                            num_idxs=S // 2, num_idxs_reg=S // 2, elem_size=H,
                            transpose=False)
                for b in range(BPC):
                    for cc in range(4):
                        nc.vector.tensor_tensor(xn[b][:, cc, :],
                                                xn[b][:, cc, :],
                                                pen(cc), op=Alu.add)
                    for k in range(8):
                        ttp = psT.tile([128, 4, 128], bf16, tag="tr", bufs=2,
                                       name=f"ept_{b}_{k}")
                        for cc in range(4):
                            nc.tensor.transpose(
                                ttp[:, cc, :],
                                xn[b][:, cc, k * 128:(k + 1) * 128], idn)
                        if k % 2 == 0:
                            nc.vector.tensor_copy(xt[b][:, k, :], ttp[:])
                        else:
                            nc.scalar.copy(xt[b][:, k, :], ttp[:])
                    for k in range(8):
                        nc.tensor.matmul(lg[32 * b:32 * (b + 1), :],
                                         oqkt(k), xt[b][:, k, :],
                                         start=(k == 0), stop=False,
                                         tile_position=(0, 32 * b),
                                         skip_group_check=True)

            # final-projection weights (needed only after the last layer)
            nc.scalar.dma_start(wot[:, 0:4], wot_d[:, 0:4])
            nc.scalar.dma_start(wot[:, 4:8], wot_d[:, 4:8])

            with tc.tile_pool(name="psO", bufs=5, space=PSUM) as psO:

                def emit_final(b):
                    # out[s, :] = x @ Wout^T  (bout added on host)
                    for cc in range(4):
                        nc.sync.dma_start(
                            xt[b][:, :, cc * 128:(cc + 1) * 128],
                            xn[b][:, cc, :], transpose=True)
                    for mg in range(4):
                        osb = wk.tile([128, OUT], bf16, tag="osb", bufs=3)
                        pA = psO.tile([128, 512], f32, tag="rs", bufs=5,
                                      name=f"fA_{b}_{mg}")
                        pB = psO.tile([128, 512], f32, tag="rs", bufs=5,
                                      name=f"fB_{b}_{mg}")
                        for k in range(8):
                            nc.tensor.matmul(
                                pA[:], xt[b][:, k, mg * 128:(mg + 1) * 128],
                                wot[:, k, 0:512], start=(k == 0),
                                stop=(k == 7))
                            nc.tensor.matmul(
                                pB[:, 0:OUT - 512],
                                xt[b][:, k, mg * 128:(mg + 1) * 128],
                                wot[:, k, 512:OUT], start=(k == 0),
                                stop=(k == 7))
                        nc.vector.tensor_copy(osb[:, 0:512], pA[:])
                        nc.scalar.copy(osb[:, 512:OUT], pB[:, 0:OUT - 512])
                        nc.sync.dma_start(out_d[b, mg], osb[:])

                def emit_out_resid(l, b, cc):
                    # n=0 half: plain matmul, drained by a DVE psum add
                    po = psO.tile([128, 512], f32, tag="rs", bufs=5,
                                  name=f"o_{l}_{cc}_{b}")
                    nc.tensor.matmul(
                        po[:],
                        opwt_cur[32 * b:32 * (b + 1),
                                 cc * 128:(cc + 1) * 128],
                        oo_cur[32 * b:32 * (b + 1), 0:512],
                        start=True, stop=True, tile_position=(32 * b, 0))
                    # n=1 half: b 0/1 plain (ACT copy + GPSIMD add), b 2/3
                    # PE identity-accumulate (ACT copy)
                    q = psO.tile([128, 512], f32, tag="rs", bufs=5,
                                 name=f"q_{l}_{cc}_{b}")
                    if b >= 2:
                        nc.tensor.matmul(q[:], idn, xn[b][:, cc, 512:1024],
                                         start=True, stop=False)
                    nc.tensor.matmul(
                        q[:],
                        opwt_cur[32 * b:32 * (b + 1),
                                 cc * 128:(cc + 1) * 128],
                        oo_cur[32 * b:32 * (b + 1), 512:1024],
                        start=(b < 2), stop=True,
                        skip_group_check=True, tile_position=(32 * b, 0))
                    return po, q

                def emit_add(b, cc, poq, l):
                    po, q = poq
                    nc.vector.tensor_tensor(xn[b][:, cc, 0:512],
                                            xn[b][:, cc, 0:512],
                                            po[:], op=Alu.add)
                    if b < 2:
                        rtmp = sm.tile([128, 512], bf16, tag="rtmp", bufs=2,
                                       name=f"rt_{l}_{cc}_{b}")
                        nc.scalar.copy(rtmp[:], q[:])
                        nc.gpsimd.tensor_tensor(xn[b][:, cc, 512:1024],
                                                xn[b][:, cc, 512:1024],
                                                rtmp[:], op=Alu.add)
                    else:
                        nc.scalar.copy(xn[b][:, cc, 512:1024], q[:])

                # ---- layers (stage-major emission, all batches per stage)
                for l in range(L):
                    # S2: shared exp (+c bias), row-sum for relw denominator
                    e_all = sm.tile([128, S], bf16, tag="e", bufs=2,
                                    name=f"e_{l}")
                    rs = sm.tile([128, 1], f32, tag="rs", bufs=2)
                    nc.scalar.activation(e_all[:], lg[:], Act.Exp,
                                         bias=c_sb[:], accum_out=rs[:])

                    # S3a: colsum via block-diag matmul; fast reciprocal
                    cs_ps = psW.tile([128, S], f32, tag="tw", name=f"cs_{l}")
                    nc.tensor.matmul(cs_ps[:], bd[:], e_all[:], start=True,
                                     stop=True)
                    rcb = sm.tile([128, S], f32, tag="rcb", bufs=2,
                                  name=f"rcb_{l}")
                    nc.vector.reciprocal_approx_fast(rcb[:], cs_ps[:])

                    # S3b: relw path: w = (e*rcs)*ops, ws = row-sum(w)
                    rcs = sm.tile([128, 1], f32, tag="rcs", bufs=2)
                    nc.vector.reciprocal(rcs[:], rs[:])
                    w_all = sm.tile([128, S], bf16, tag="w", bufs=2,
                                    name=f"w_{l}")
                    ws = sm.tile([128, 1], f32, tag="ws", bufs=2)
                    nc.vector.scalar_tensor_tensor(w_all[:], e_all[:],
                                                   rcs[:], ops_s,
                                                   op0=Alu.mult,
                                                   op1=Alu.mult,
                                                   accum_out=ws[:])

                    # S3c: opw numerator: opwt = e * (1/colsum broadcast)
                    opwt = sm.tile([128, S], bf16, tag="opwt", bufs=2,
                                   name=f"opwt_{l}")
                    nc.vector.tensor_tensor(opwt[:], e_all[:], rcb[:],
                                            op=Alu.mult)

                    # S4: w^T via xbar, then t (col-tiled 4-way)
                    wt_sb = sm.tile([128, 4, 128], bf16, tag="wt", bufs=2,
                                    name=f"wt_{l}")
                    nc.sync.dma_start(wt_sb[:], w_all[:], transpose=True)
                    t_ps = [psW.tile([128, 512], f32, tag="tw",
                                     name=f"t_{l}_{n}") for n in range(2)]
                    for cc in range(4):
                        for n in range(2):
                            for b in range(BPC):
                                nc.tensor.matmul(
                                    t_ps[n][32 * b:32 * (b + 1), :],
                                    wt_sb[:, cc, 32 * b:32 * (b + 1)],
                                    xn[b][:, cc, n * 512:(n + 1) * 512],
                                    start=(cc == 0), stop=(cc == 3),
                                    tile_position=(0, 32 * b))
                    t_sb = sm.tile([128, H], bf16, tag="tsb", bufs=2,
                                   name=f"t_{l}")
                    nc.scalar.copy(t_sb[:, 0:512], t_ps[0][:])
                    nc.scalar.copy(t_sb[:, 512:], t_ps[1][:])

                    # S5: t^T on PE (keeps PE warm), then oo (M=128 batched)
                    tt_sb = sm.tile([128, 8, 128], bf16, tag="tt", bufs=2,
                                    name=f"tt_{l}")
                    for g in range(2):
                        trp = psW.tile([128, 4, 128], bf16, tag="tw",
                                       name=f"tr_{l}_{g}")
                        for k in range(4):
                            nc.tensor.transpose(
                                trp[:, k, :],
                                t_sb[:, (g * 4 + k) * 128:
                                     (g * 4 + k + 1) * 128], idn)
                        nc.vector.tensor_copy(tt_sb[:, g * 4:(g + 1) * 4],
                                              trp[:])
                    oo_ps = [psW.tile([128, 512], f32, tag="tw",
                                      name=f"oo_{l}_{n}") for n in range(2)]
                    for n in range(2):
                        for k in range(8):
                            nc.tensor.matmul(oo_ps[n][:], tt_sb[:, k, :],
                                             wvt[:, k,
                                                 n * 512:(n + 1) * 512],
                                             start=(k == 0), stop=(k == 7))
                    oo_sb = sm.tile([128, H], bf16, tag="oo", bufs=2,
                                    name=f"oo_{l}")
                    for n in range(2):
                        nc.vector.scalar_tensor_tensor(
                            oo_sb[:, n * 512:(n + 1) * 512], bvb(n), ws[:],
                            oo_ps[n][:], op0=Alu.mult, op1=Alu.add)

                    # S5b: incremental logits for the next layer:
                    # G = t @ A2 + ws x g0;  lg += G^T-strips @ opwt
                    if l < L - 1:
                        g_full = psW.tile([128, 512], f32, tag="tw",
                                          name=f"g_{l}")
                        g_ps = g_full[:, 0:32]
                        for k in range(8):
                            nc.tensor.matmul(g_ps, tt_sb[:, k, :], a2p(k),
                                             start=(k == 0), stop=(k == 7))
                        g_sb = sm.tile([128, 32], bf16, tag="gsb", bufs=2,
                                       name=f"gsb_{l}")
                        nc.vector.scalar_tensor_tensor(g_sb[:], g0b, ws[:],
                                                       g_ps, op0=Alu.mult,
                                                       op1=Alu.add)
                        for b in range(BPC):
                            nc.tensor.matmul(lg[32 * b:32 * (b + 1), :],
                                             g_sb[32 * b:32 * (b + 1), :],
                                             opwt[32 * b:32 * (b + 1), :],
                                             start=False, stop=(l == L - 2),
                                             tile_position=(32 * b, 32 * b),
                                             skip_group_check=True)

                    # S6: out + residual, one K=32 N=1024 bf16 matmul and
                    # one [128,1024] add per (b, cc).  Last layer goes
                    # batch-major so the final projection overlaps.
                    opwt_cur, oo_cur = opwt, oo_sb
                    for cc in range(4):
                        pos = [emit_out_resid(l, b, cc)
                               for b in range(BPC)]
                        for b in range(BPC):
                            emit_add(b, cc, pos[b], l)
                    if l == L - 1:
                        for b in range(BPC):
                            emit_final(b)

    nc.compile()
    return nc


def _prep_inputs(inputs):
    """Host-side sharding + weight-only preprocessing. in_maps for 8 cores."""
    tokens = np.asarray(inputs["tokens"]).astype(np.int64)
    word_emb = np.ascontiguousarray(np.asarray(inputs["word_emb"], np.float32))
    Wv = np.asarray(inputs["Wv"], np.float32)
    bv = np.asarray(inputs["bv"], np.float32)
    Wk = np.asarray(inputs["Wk"], np.float32)
    bk = np.asarray(inputs["bk"], np.float32)
    Wq = np.asarray(inputs["Wq_op"], np.float32)
    bq = np.asarray(inputs["bq_op"], np.float32)
    ops = np.asarray(inputs["operators"], np.float32)
    Wout = np.asarray(inputs["Wout"], np.float32)

    scale = 1.0 / math.sqrt(H)
    oq = ops @ Wq.T + bq                      # [O, H]
    oqkT = (Wk.T @ oq.T) * scale              # [H, O]
    c = (bk @ oq.T) * scale                   # [O]
    A2 = Wv.T @ oqkT                          # [H, O]
    g0 = bv @ oqkT                            # [O]

    def chunked(a):
        D, N = a.shape
        return np.ascontiguousarray(a.reshape(8, 128, N).transpose(1, 0, 2))

    def chunk_pad32(a16):                     # [H, O] -> [128, 8*32]
        out = np.zeros((128, 8, 32), np.float32)
        out[:, :, :O] = chunked(a16)
        return out.reshape(128, 8 * 32)

    pe = _sinusoidal_pos_emb(S, H)            # [S, H]
    pen = np.ascontiguousarray(
        pe.reshape(4, 128, H).transpose(1, 0, 2)).reshape(128, 4 * H)

    c_strip = np.full((128, 1), -30.0, np.float32)
    ops_strip = np.zeros((128, 512), np.float32)
    for b4 in range(4):
        c_strip[32 * b4:32 * b4 + O, 0] = c
        ops_strip[32 * b4:32 * b4 + O] = ops

    bd = np.zeros((128, 128), np.float32)
    for b4 in range(4):
        bd[32 * b4:32 * (b4 + 1), 32 * b4:32 * (b4 + 1)] = 1.0

    g0p = np.zeros((1, 32), np.float32)
    g0p[0, :O] = g0

    wpk = np.concatenate([
        pen,
        np.tile(bv.reshape(1, H), (128, 1)),
        ops_strip,
        chunk_pad32(oqkT),
        np.eye(128, dtype=np.float32),
        bd,
        chunk_pad32(A2),
        np.tile(g0p, (128, 1)),
    ], axis=1).astype(BF16)
    assert wpk.shape == (128, WC), wpk.shape

    common = {
        "emb": word_emb.astype(BF16),
        "wpk": wpk,
        "cst": c_strip,
        "wvt": chunked(Wv.T.copy()).astype(BF16),
        "wot": chunked(Wout.T.copy()).astype(BF16),
    }

    in_maps = []
    for cid in range(NCORES):
        toks = tokens[cid * BPC:(cid + 1) * BPC]
        idx = np.zeros((128, BPC, S // 16), np.int16)
        for b in range(BPC):
            t16 = toks[b].reshape(S // 16, 16).T.astype(np.int16)
            idx[:, b, :] = np.tile(t16, (8, 1))
        in_maps.append({**common, "tok": idx})
    return in_maps


def kernel(**inputs):
    from concourse.bass_utils import run_bass_kernel_spmd

    if "nc" not in _cache:
        _cache["nc"] = _build_program()
    nc = _cache["nc"]

    in_maps = _prep_inputs(inputs)
    res = run_bass_kernel_spmd(nc, in_maps, list(range(NCORES)))
    outs = []
    for cid in range(NCORES):
        o = res.results[cid]["out"]  # [BPC, 4, 128, OUT] bf16
        outs.append(np.asarray(o, dtype=np.float32).reshape(BPC, S, OUT))
    bout = np.asarray(inputs["bout"], np.float32).reshape(1, 1, OUT)
    return np.concatenate(outs, axis=0) + bout
